# revision 2
# baseline (speedup 1.0000x reference)
"""Trainium2 Bass kernel for nn_MultiHeadAttention_32066225832689.

Reference computation (B=2, S=2048, D=1024, fp32):
    q = relu(x @ Wq + bq); k = relu(x @ Wk + bk); v = relu(x @ Wv + bv)
    e = (q @ k^T) / sqrt(D);  e -= 10000 * causal_mask
    attn = softmax(e);  y = relu((attn @ v) @ Wo + bo)
Biases are all zeros by problem spec (fill: zeros) and are ignored.

Sharding over 8 NeuronCores: batch (2) x rank (4).  Rank r of a batch
group owns:
  - K/V projection for token rows [512r, 512r+512) (data-parallel),
    exchanged via ONE 8-core AllGather per tensor (Shared-output fast
    path; the 4-rank grouped AllGather takes the slow fold_n=2 ring and
    measured ~77us vs ~14us here).  The 8-core gather mixes both
    batch groups' chunks, so each core picks its batch's four chunks
    out of the gathered buffer with indirect DMAs whose row indices are
    host-provided per-core data -- the SPMD program stays identical on
    all cores.
  - Query chunks {r, r+4, r+8, r+12} (128 rows each).  Chunk c needs
    key chunks 0..c//4, so every rank processes blocks with 1,2,3,4
    key chunks of 512 -- a balanced, rank-uniform causal workload.

The whole data plane runs in bfloat16 (weights, x^T, K^T, V, Q^T, P):
same 1 cycle/row PE rate as fp32r but half the DMA/collective bytes
and SBUF footprint.  Score/softmax/output accumulation stays fp32.
Numpy modeling of this exact rounding placement gives rel err ~4e-3
against the fp32 reference (budget 2e-2).

x^T is pre-transposed on the host (free), removing the on-device PE
transpose stage of v1.  1/rowsum is folded into the final relu's
per-partition scale.  Weights/x stream on the sync HWDGE ring; bounce
writes + output stores on the scalar ring; collectives + gathered-chunk
indirect loads on the gpsimd ring -- so no input stream ever waits on a
collective.
"""

import sys

sys.path.insert(0, "/opt/trn_rl_repo")

import numpy as np
import ml_dtypes

import concourse.bass as bass
import concourse.mybir as mybir
from concourse import tile
from concourse.bass_utils import run_bass_kernel_spmd

F32 = mybir.dt.float32
BF16 = mybir.dt.bfloat16
I32 = mybir.dt.int32

B, S, D = 2, 2048, 1024
NEG = 10000.0
SCALE = 1.0 / 32.0  # 1/sqrt(D)

# ---------------------------------------------------------------------------
# Post-scheduling pass: split multi-wait instructions into NOP chains.
# The pinned walrus codegen accepts only one embedded sync-wait per
# instruction on most engine instruction formats; Tile's semaphore
# assignment freely emits several.  Rewrite each instruction with k>1
# waits into (k-1) same-engine NoOps carrying one wait each, inserted
# immediately before it (same engine program order => semantics kept).
# ---------------------------------------------------------------------------
_WSPLIT_CTR = [0]


def _split_waits(nc, max_waits=1):
    n = 0
    for f in nc.m.functions:
        for blk in f.blocks:
            out = []
            for inst in blk.instructions:
                si = inst.sync_info
                if si is not None and len(si.on_wait) > max_waits:
                    waits = list(si.on_wait)
                    for w in waits[:-max_waits]:
                        _WSPLIT_CTR[0] += 1
                        nop = mybir.InstNoOp(name=f"WSPLIT-{_WSPLIT_CTR[0]}")
                        nop.engine = inst.engine
                        nop.sync_info = mybir.SyncInfo(on_wait=[w], on_update=[])
                        out.append(nop)
                    inst.sync_info = mybir.SyncInfo(
                        on_wait=waits[-max_waits:], on_update=list(si.on_update)
                    )
                    n += 1
                out.append(inst)
            blk.instructions = out
    return n


# ---------------------------------------------------------------------------
# Kernel program (identical on all 8 cores)
# ---------------------------------------------------------------------------


def _build_program(timing=False, reps=1, no_cc=False):
    """timing=True builds a single-core variant (no collective; gathered
    K^T/V reads redirected to the local bounce buffer) with identical
    instruction mix/volume, for TimelineSim cost-model analysis.
    reps>1 emits the whole kernel body that many times back-to-back
    (benchmarking: amortizes the per-dispatch overhead)."""
    nc = bass.Bass(
        "TRN2", target_bir_lowering=False, debug=False,
        num_devices=1 if timing else 8,
    )

    x_kvT = nc.dram_tensor("x_kvt", [D, 512], BF16, kind="ExternalInput")
    x_qT = nc.dram_tensor("x_qt", [D, 512], BF16, kind="ExternalInput")
    wq_in = nc.dram_tensor("wq", [D, D], BF16, kind="ExternalInput")
    wk_in = nc.dram_tensor("wk", [D, D], BF16, kind="ExternalInput")
    wv_in = nc.dram_tensor("wv", [D, D], BF16, kind="ExternalInput")
    wo_in = nc.dram_tensor("wo", [D, D], BF16, kind="ExternalInput")
    mask_in = nc.dram_tensor("mask", [128, 512], F32, kind="ExternalInput")
    ident_in = nc.dram_tensor("ident", [128, 128], F32, kind="ExternalInput")
    gidx_in = nc.dram_tensor("gidx", [128, 8], I32, kind="ExternalInput")
    y_out = nc.dram_tensor("y_out", [512, D], F32, kind="ExternalOutput")

    with tile.TileContext(nc) as tc:
        for _rep in range(reps):
            _emit(nc, tc, x_kvT, x_qT, wq_in, wk_in, wv_in, wo_in, mask_in,
                  ident_in, gidx_in, y_out, timing or no_cc)

    _split_waits(nc)
    return nc


SECTIONS = []


def _emit(nc, tc, x_kvT, x_qT, wq_in, wk_in, wv_in, wo_in, mask_in, ident_in,
          gidx_in, y_out, timing):
    del SECTIONS[:]

    def mark(label):
        try:
            SECTIONS.append((nc.next_id(), label))
        except Exception:
            pass

    Relu = mybir.ActivationFunctionType.Relu
    Exp = mybir.ActivationFunctionType.Exp
    AX = mybir.AxisListType.X

    pools = []

    def pool(name, bufs, space="SBUF"):
        p = tc.alloc_tile_pool(name=name, bufs=bufs, space=space)
        pools.append(p)
        return p

    # ----- long-lived pools -----
    const_p = pool("const", 1)
    qt_p = pool("qt", 1)
    wo_p = pool("wo", 1)
    e_p = pool("e", 1)
    pt_p = pool("pt", 2)
    y_p = pool("y", 1)
    yt_p = pool("yt", 1)
    out_p = pool("out", 1)
    st_p = pool("st", 2)
    res_p = pool("res", 1)
    dram_p = pool("dram", 1, space="DRAM")

    ident_t = const_p.tile([128, 128], F32, tag="ident")
    nc.sync.dma_start(ident_t[:], ident_in.ap())
    mask_t = const_p.tile([128, 512], F32, tag="mask")
    nc.sync.dma_start(mask_t[:], mask_in.ap())
    gidx_t = const_p.tile([128, 8], I32, tag="gidx")
    nc.sync.dma_start(gidx_t[:], gidx_in.ap())

    qt_t = qt_p.tile([128, 4096], BF16, tag="qt")   # [dout d-tile, 512 q-tok]
    wo_t = wo_p.tile([128, 8192], BF16, tag="wo")   # [din d-tile, 1024 dout]

    # gathered K^T / V residency: all 4 chunks (incl. own) come back from
    # the gathered buffer via indirect DMA -- the own-chunk's gather row
    # position is per-core data, so the program stays core-agnostic.
    kt_res = [res_p.tile([128, 4096], BF16, tag=f"kt{g}", name=f"kt_res{g}")
              for g in range(4)]
    v_res = [res_p.tile([128, 4096], BF16, tag=f"v{g}", name=f"v_res{g}")
             for g in range(4)]

    # collective bounce buffers (DRAM pool tiles -> Tile tracks deps).
    # Shard layout: one core's chunk = [128 rows, 4096 bf16] (8 KiB rows),
    # so a gathered chunk reload is ONE per-partition row-gather DMA.
    kt_in = dram_p.tile([128, 4096], BF16, tag="kt_in")
    v_in = dram_p.tile([128, 4096], BF16, tag="v_in")
    if not timing:
        kt_gath = dram_p.tile([1024, 4096], BF16, tag="kt_gath",
                              addr_space="Shared")
        v_gath = dram_p.tile([1024, 4096], BF16, tag="v_gath",
                             addr_space="Shared")
    else:
        kt_gath, v_gath = kt_in, v_in

    def load_chunk(dst, src, col):
        nc.gpsimd.indirect_dma_start(
            out=dst[:, :],
            out_offset=None,
            in_=src[:, :],
            in_offset=bass.IndirectOffsetOnAxis(ap=gidx_t[:, col:col + 1],
                                                axis=0),
        )

    # =====================================================================
    # Projections: K^T own -> AllGather -> V own -> AllGather -> Q^T + Wo
    # =====================================================================
    with tc.tile_pool(name="pP", bufs=1) as pp, \
         tc.tile_pool(name="wstream", bufs=8) as wsp, \
         tc.tile_pool(name="ps_pmm", bufs=8, space="PSUM") as ps_mm:

        xt_kv = pp.tile([128, 4096], BF16, tag="xt_kv")
        xt_q = pp.tile([128, 4096], BF16, tag="xt_q")
        kt_own = pp.tile([128, 4096], BF16, tag="kt_own")
        v_own = pp.tile([128, 4096], BF16, tag="v_own")

        mark("xt_kv")
        xkv3 = xt_kv.rearrange("p (d b) -> p d b", b=512)
        for d in range(8):
            nc.sync.dma_start(xkv3[:, d, :],
                              x_kvT.ap()[128 * d:128 * (d + 1), :])

        # ---- K^T own: kt_own[p, 512m+tok] = relu(K^T)[128m+p, tok]
        mark("kT")
        for half in range(2):
            mms = [ps_mm.tile([128, 512], F32, tag="mm", name=f"mmk{half}_{i}")
                   for i in range(4)]
            for d in range(8):
                wk_d = wsp.tile([128, 512], BF16, tag="w", name=f"wk{half}{d}")
                nc.sync.dma_start(
                    wk_d[:], wk_in.ap()[128 * d:128 * (d + 1),
                                        512 * half:512 * (half + 1)])
                for mi in range(4):
                    nc.tensor.matmul(
                        mms[mi][:],
                        wk_d[:, 128 * mi:128 * (mi + 1)],
                        xt_kv[:, 512 * d:512 * (d + 1)],
                        start=(d == 0), stop=(d == 7),
                    )
            for mi in range(4):
                m = 4 * half + mi
                nc.scalar.activation(kt_own[:, 512 * m:512 * (m + 1)],
                                     mms[mi][:], Relu)
        nc.scalar.dma_start(kt_in[:, :], kt_own[:, :])

        mark("cc_kt")
        if not timing:
            nc.gpsimd.collective_compute(
                "AllGather",
                mybir.AluOpType.bypass,
                replica_groups=[[0, 1, 2, 3, 4, 5, 6, 7]],
                ins=[kt_in[:, :]],
                outs=[kt_gath[:, :]],
            )

        # ---- V own: v_own[p, 1024t+dv] = relu(V)[128t+p, dv]
        mark("V")
        for h in range(2):
            mms = [ps_mm.tile([128, 512], F32, tag="mm", name=f"mmv{h}_{i}")
                   for i in range(4)]
            for d in range(8):
                wv_d = wsp.tile([128, 512], BF16, tag="w", name=f"wv{h}{d}")
                nc.sync.dma_start(
                    wv_d[:], wv_in.ap()[128 * d:128 * (d + 1),
                                        512 * h:512 * (h + 1)])
                for t in range(4):
                    nc.tensor.matmul(
                        mms[t][:],
                        xt_kv[:, 512 * d + 128 * t:512 * d + 128 * (t + 1)],
                        wv_d[:],
                        start=(d == 0), stop=(d == 7),
                    )
            for t in range(4):
                nc.scalar.activation(
                    v_own[:, 1024 * t + 512 * h:1024 * t + 512 * (h + 1)],
                    mms[t][:], Relu)
        nc.scalar.dma_start(v_in[:, :], v_own[:, :])

        mark("cc_v")
        if not timing:
            nc.gpsimd.collective_compute(
                "AllGather",
                mybir.AluOpType.bypass,
                replica_groups=[[0, 1, 2, 3, 4, 5, 6, 7]],
                ins=[v_in[:, :]],
                outs=[v_gath[:, :]],
            )

        mark("xt_q")
        xq3 = xt_q.rearrange("p (d b) -> p d b", b=512)
        for d in range(8):
            nc.sync.dma_start(xq3[:, d, :],
                              x_qT.ap()[128 * d:128 * (d + 1), :])

        # ---- Q^T own (scaled by 1/sqrt(D))
        mark("qT")
        for half in range(2):
            mms = [ps_mm.tile([128, 512], F32, tag="mm", name=f"mmq{half}_{i}")
                   for i in range(4)]
            for d in range(8):
                wq_d = wsp.tile([128, 512], BF16, tag="w", name=f"wq{half}{d}")
                nc.sync.dma_start(
                    wq_d[:], wq_in.ap()[128 * d:128 * (d + 1),
                                        512 * half:512 * (half + 1)])
                for mi in range(4):
                    nc.tensor.matmul(
                        mms[mi][:],
                        wq_d[:, 128 * mi:128 * (mi + 1)],
                        xt_q[:, 512 * d:512 * (d + 1)],
                        start=(d == 0), stop=(d == 7),
                    )
            for mi in range(4):
                m = 4 * half + mi
                nc.scalar.activation(qt_t[:, 512 * m:512 * (m + 1)],
                                     mms[mi][:], Relu, scale=SCALE)

        mark("woload")
        for d in range(8):
            nc.sync.dma_start(wo_t[:, 1024 * d:1024 * (d + 1)],
                              wo_in.ap()[128 * d:128 * (d + 1), :])

        # gathered chunk loads (gpsimd ring, behind the AG triggers; K
        # chunks first -- scores need them before AV needs V chunks)
        mark("kvload")
        for g in range(4):
            load_chunk(kt_res[g], kt_gath, g)
        for g in range(4):
            load_chunk(v_res[g], v_gath, 4 + g)

    # =====================================================================
    # Attention + output projection, software-pipelined across blocks
    # =====================================================================
    with tc.tile_pool(name="ps_cmm", bufs=2, space="PSUM") as ps_mm, \
         tc.tile_pool(name="ps_ctr", bufs=2, space="PSUM") as ps_tr, \
         tc.tile_pool(name="ps_y", bufs=1, space="PSUM") as ps_y, \
         tc.tile_pool(name="ps_yt", bufs=1, space="PSUM") as ps_yt:

        st = {}

        def emit_e(i):
            mark(f"e{i}")
            if i == 0:
                # the [128,1024] output-staging slot is idle until tail0 and
                # block0's scores die before tail0 -- reuse it so block1's
                # copies never wait on block0's transposes.
                e_t = out_p.tile([128, 1024], F32, tag="osb", name="e0")[:, 0:512]
            else:
                e_t = e_p.tile([128, 512 * (i + 1)], F32,
                               tag=("eA" if i < 2 else "eB"), name=f"e{i}")
            st[i] = {"e": e_t}
            for g in range(i + 1):
                mm = ps_mm.tile([128, 512], F32, tag="mm", name=f"mme{i}{g}")
                ktg = kt_res[g].rearrange("p (d b) -> p d b", b=512)
                for d in range(8):
                    nc.tensor.matmul(
                        mm[:],
                        qt_t[:, 512 * d + 128 * i:512 * d + 128 * (i + 1)],
                        ktg[:, d, :],
                        start=(d == 0), stop=(d == 7),
                    )
                if g == i:
                    nc.vector.tensor_add(e_t[:, 512 * g:512 * (g + 1)],
                                         mm[:], mask_t[:])
                else:
                    nc.vector.tensor_copy(e_t[:, 512 * g:512 * (g + 1)], mm[:])

        def emit_softmax(i):
            mark(f"sm{i}")
            e_t = st[i]["e"]
            W = 512 * (i + 1)
            negmax = st_p.tile([128, 1], F32, tag="negmax", name=f"nm{i}")
            nc.vector.reduce_max(negmax[:], e_t[:, 0:W], axis=AX, negate=True)
            rowsum = st_p.tile([128, 1], F32, tag="rowsum", name=f"rs{i}")
            nc.scalar.activation(e_t[:, 0:W], e_t[:, 0:W], Exp,
                                 bias=negmax[:], scale=1.0, accum_out=rowsum[:])
            rinv = st_p.tile([128, 1], F32, tag="rinv", name=f"ri{i}")
            nc.vector.reciprocal(rinv[:], rowsum[:])
            st[i]["rinv"] = rinv

        def emit_trav(i):
            mark(f"av{i}")
            e_t = st[i]["e"]
            yps = ps_y.tile([128, 1024], F32, tag="yacc", name=f"y{i}")
            st[i]["yps"] = yps
            for g in range(i + 1):
                trp = ps_tr.tile([128, 512], F32, tag="ctr", name=f"ctr{i}{g}")
                for j in range(4):
                    nc.tensor.transpose(
                        trp[:, 128 * j:128 * (j + 1)],
                        e_t[:, 512 * g + 128 * j:512 * g + 128 * (j + 1)],
                        ident_t[:],
                    )
                pt_t = pt_p.tile([128, 512], BF16, tag="pt", name=f"pt{i}{g}")
                nc.vector.tensor_copy(pt_t[:], trp[:])
                vg = v_res[g].rearrange("p (t b) -> p t b", b=1024)
                for j in range(4):
                    for h in range(2):
                        nc.tensor.matmul(
                            yps[:, 512 * h:512 * (h + 1)],
                            pt_t[:, 128 * j:128 * (j + 1)],
                            vg[:, j, 512 * h:512 * (h + 1)],
                            start=(g == 0 and j == 0),
                            stop=(g == i and j == 3),
                        )

        def emit_tail(i):
            # y stays unnormalized; 1/rowsum is applied as the per-partition
            # scale of the final relu (relu(a*c) = relu(a)*c for c > 0).
            mark(f"tail{i}")
            y_t = y_p.tile([128, 1024], F32, tag="ysb", name=f"ysb{i}")
            nc.vector.tensor_copy(y_t[:], st[i]["yps"][:])
            ytp = ps_yt.tile([128, 1024], F32, tag="ytp", name=f"ytp{i}")
            for d in range(8):
                nc.tensor.transpose(
                    ytp[:, 128 * d:128 * (d + 1)],
                    y_t[:, 128 * d:128 * (d + 1)],
                    ident_t[:],
                )
            yt_t = yt_p.tile([128, 1024], BF16, tag="ytsb", name=f"ytsb{i}")
            nc.vector.tensor_copy(yt_t[:], ytp[:])
            o_t = out_p.tile([128, 1024], F32, tag="osb", name=f"osb{i}")
            for h in range(2):
                mm = ps_mm.tile([128, 512], F32, tag="mm", name=f"mmo{i}{h}")
                for d in range(8):
                    nc.tensor.matmul(
                        mm[:],
                        yt_t[:, 128 * d:128 * (d + 1)],
                        wo_t[:, 1024 * d + 512 * h:1024 * d + 512 * (h + 1)],
                        start=(d == 0), stop=(d == 7),
                    )
                nc.scalar.activation(o_t[:, 512 * h:512 * (h + 1)], mm[:], Relu,
                                     scale=st[i]["rinv"][:])
            nc.scalar.dma_start(y_out.ap()[128 * i:128 * (i + 1), :], o_t[:])

        # pipelined emission: PE fills softmax bubbles with the next
        # block's score matmuls.
        emit_e(0)
        emit_softmax(0)
        emit_e(1)
        emit_trav(0)
        emit_tail(0)
        emit_softmax(1)
        emit_e(2)
        emit_trav(1)
        emit_tail(1)
        emit_softmax(2)
        emit_e(3)
        emit_trav(2)
        emit_tail(2)
        emit_softmax(3)
        emit_trav(3)
        emit_tail(3)

    mark("end")
    for p in reversed(pools):
        p.release()


_PROGRAM_CACHE = {}


def _get_program():
    if "nc" not in _PROGRAM_CACHE:
        _PROGRAM_CACHE["nc"] = _build_program()
    return _PROGRAM_CACHE["nc"]


# ---------------------------------------------------------------------------
# Host-side entry point
# ---------------------------------------------------------------------------


def _bf16(a):
    return np.asarray(np.asarray(a, dtype=np.float32),
                      dtype=ml_dtypes.bfloat16)


def _make_mask(r):
    i = np.arange(128)[:, None]
    j = np.arange(512)[None, :]
    return np.where(j > 128 * r + i, np.float32(-NEG), np.float32(0.0))


def _make_in_maps(x, Wq, Wk, Wv, Wo):
    x = np.asarray(x, dtype=np.float32)
    wq = _bf16(Wq)
    wk = _bf16(Wk)
    wv = _bf16(Wv)
    wo = _bf16(Wo)
    ident = np.eye(128, dtype=np.float32)
    p = np.arange(128, dtype=np.int32)

    in_maps = []
    for core in range(8):
        b, r = divmod(core, 4)
        xb = x[b]
        x_kvT = np.ascontiguousarray(_bf16(xb[512 * r:512 * (r + 1)]).T)
        chunks = [r, r + 4, r + 8, r + 12]
        x_q = np.concatenate([xb[128 * c:128 * (c + 1)] for c in chunks],
                             axis=0)
        x_qT = np.ascontiguousarray(_bf16(x_q).T)
        gidx = np.empty((128, 8), dtype=np.int32)
        for g in range(4):
            gidx[:, g] = 128 * (4 * b + g) + p
            gidx[:, 4 + g] = 128 * (4 * b + g) + p
        in_maps.append({
            "x_kvt": x_kvT, "x_qt": x_qT,
            "wq": wq, "wk": wk, "wv": wv, "wo": wo,
            "mask": _make_mask(r), "ident": ident, "gidx": gidx,
        })
    return in_maps


def kernel(x, Wq, bq, Wk, bk, Wv, bv, Wo, bo, _bench=None):
    nc = _get_program()
    in_maps = _make_in_maps(x, Wq, Wk, Wv, Wo)

    kwargs = dict(_bench or {})
    res = run_bass_kernel_spmd(nc, in_maps, list(range(8)), **kwargs)

    out = np.empty((B, S, D), dtype=np.float32)
    for core in range(8):
        b, r = divmod(core, 4)
        yo = res.results[core]["y_out"]
        for i, c in enumerate([r, r + 4, r + 8, r + 12]):
            out[b, 128 * c:128 * (c + 1), :] = yo[128 * i:128 * (i + 1), :]
    if _bench is not None:
        kernel.last_result = res
    return out


kernel.last_result = None


# ---------------------------------------------------------------------------
# Benchmarking helper (used by test.py only): runs the kernel repeatedly
# through a persistent jitted PJRT executable with device-resident inputs,
# so per-call wall time approximates dispatch-overhead + HW exec time.
# ---------------------------------------------------------------------------


def make_runner(nc, in_maps):
    import jax
    from jax.sharding import Mesh, PartitionSpec, NamedSharding
    from jax.experimental.shard_map import shard_map
    from concourse.bass2jax import (
        _bass_exec_p, install_neuronx_cc_hook, partition_id_tensor,
    )

    install_neuronx_cc_hook()
    n_cores = len(in_maps)
    in_names, out_names, out_avals, zero_outs = [], [], [], []
    pname = nc.partition_id_tensor.name if nc.partition_id_tensor else None
    for alloc in nc.m.functions[0].allocations:
        if not isinstance(alloc, mybir.MemoryLocationSet):
            continue
        name = alloc.memorylocations[0].name
        if alloc.kind == "ExternalInput":
            if name != pname:
                in_names.append(name)
        elif alloc.kind == "ExternalOutput":
            shape = tuple(alloc.tensor_shape)
            dtype = mybir.dt.np(alloc.dtype)
            out_names.append(name)
            out_avals.append(jax.core.ShapedArray(shape, dtype))
            zero_outs.append(np.zeros(shape, dtype))
    n_params = len(in_names)
    all_in = list(in_names) + list(out_names)
    if pname:
        all_in.append(pname)

    def _body(*args):
        operands = list(args)
        if pname is not None:
            operands.append(partition_id_tensor())
        return tuple(_bass_exec_p.bind(
            *operands, out_avals=tuple(out_avals), in_names=tuple(all_in),
            out_names=tuple(out_names), lowering_input_output_aliases=(),
            sim_require_finite=True, sim_require_nnan=True, nc=nc))

    devices = jax.devices()[:n_cores]
    mesh = Mesh(np.asarray(devices), ("core",))
    specs_in = (PartitionSpec("core"),) * (n_params + len(out_names))
    specs_out = (PartitionSpec("core"),) * len(out_names)
    fn = jax.jit(shard_map(_body, mesh=mesh, in_specs=specs_in,
                           out_specs=specs_out, check_rep=False),
                 keep_unused=True)
    sh = NamedSharding(mesh, PartitionSpec("core"))
    concat_in = [np.concatenate([np.asarray(m[n]) for m in in_maps], axis=0)
                 for n in in_names]
    concat_zero = [np.zeros((n_cores * z.shape[0], *z.shape[1:]), z.dtype)
                   for z in zero_outs]
    dev_in = [jax.device_put(a, sh) for a in concat_in]
    dev_zero = [jax.device_put(a, sh) for a in concat_zero]
    return fn, dev_in, dev_zero, out_names


def bench_hw(inputs, iters=60, trials=3):
    """Pipelined-dispatch wall-clock per call for the real 8-core program
    and for a trivial null program; the difference estimates HW exec time."""
    import time
    import jax

    in_maps = _make_in_maps(inputs["x"], inputs["Wq"], inputs["Wk"],
                            inputs["Wv"], inputs["Wo"])

    def null_program():
        nnc = bass.Bass("TRN2", target_bir_lowering=False, debug=False,
                        num_devices=8)
        xi = nnc.dram_tensor("xn", [128, 128], F32, kind="ExternalInput")
        yo = nnc.dram_tensor("yn", [128, 128], F32, kind="ExternalOutput")
        with tile.TileContext(nnc) as tcc:
            with tcc.tile_pool(name="s", bufs=1) as pl:
                t = pl.tile([128, 128], F32, name="t0")
                nnc.sync.dma_start(t[:], xi.ap())
                nnc.sync.dma_start(yo.ap(), t[:])
        _split_waits(nnc)
        return nnc, [{"xn": np.zeros((128, 128), np.float32)}] * 8

    def measure(fn, di, dz):
        out = fn(*di, *dz)
        jax.block_until_ready(out)
        best = float("inf")
        for _ in range(trials):
            t0 = time.perf_counter()
            outs = [fn(*di, *dz) for _ in range(iters)]
            jax.block_until_ready(outs)
            dt = (time.perf_counter() - t0) / iters
            best = min(best, dt)
        return best

    nnc, null_maps = null_program()
    fn0, di0, dz0, _ = make_runner(nnc, null_maps)
    t_null = measure(fn0, di0, dz0)
    fn1, di1, dz1, _ = make_runner(_get_program(), in_maps)
    t_full = measure(fn1, di1, dz1)
    return t_full, t_null


# revision 29
# speedup vs baseline: 1.6938x; 1.6938x over previous
"""Trainium2 Bass kernel for nn_MultiHeadAttention_32066225832689.

Reference computation (B=2, S=2048, D=1024, fp32):
    q = relu(x @ Wq + bq); k = relu(x @ Wk + bk); v = relu(x @ Wv + bv)
    e = (q @ k^T) / sqrt(D);  e -= 10000 * causal_mask
    attn = softmax(e);  y = relu((attn @ v) @ Wo + bo)
Biases are all zeros by problem spec (fill: zeros) and are ignored.

Sharding over 8 NeuronCores: batch (2) x rank (4).  Rank r of a batch
group owns:
  - K/V projection for token rows [512r, 512r+512) (data-parallel).
    Both tensors are exchanged in ONE fused 8-core AllGather with a
    Shared-address-space output (the fast collective path; Shared
    output needs >4-core groups, and the 4-rank grouped AllGather
    takes the slow fold_n=2 ring: ~77us/AG vs the whole fused
    exchange costing ~60us steady-state here.  Two separate 8-core
    AGs serialize on ncfw and measured 45us slower than one fused).
    The 8-core gather mixes both batch groups' chunks, so each core
    picks its batch's four chunks out of the gathered buffer with
    per-partition row-gather indirect DMAs whose row indices are
    host-provided per-core data -- the SPMD program stays identical
    on all cores.  (Static conditional DMAs gated on a host flag via
    values_load would avoid the SWDGE path, but the pinned walrus
    cannot encode the bounds-check register ISA: "ISA wrong length".)
  - Query chunks {r, r+4, r+8, r+12} (128 rows each).  Chunk c needs
    key chunks 0..c//4, so every rank processes blocks with 1,2,3,4
    key chunks of 512 -- a balanced, rank-uniform causal workload.

The whole data plane runs in bfloat16 (weights, x^T, K^T, V, Q^T, P):
same 1 cycle/row PE rate as fp32r but half the DMA/collective bytes
and SBUF footprint.  Scores/softmax/output accumulation stay fp32.
Numpy modeling of this rounding placement predicts rel err ~4e-3 and
hardware measures 2.0e-3 against the fp32 reference (budget 2e-2).
fp8e4m3 K/V was evaluated numerically at 3.7e-2 -- over budget.

x^T is pre-transposed on the host (free), removing v1's on-device PE
transpose stage.  The row max is accumulated incrementally per score
chunk (hidden behind the next chunk's matmuls) and 1/rowsum is folded
into the final relu's per-partition scale.  Weights/x stream on the
sync HWDGE ring (first x^T slice and first wk slice lead; consts
trail), bounce writes + output stores on the scalar ring, collective +
gathered-chunk loads on the gpsimd ring -- no input stream ever waits
on the collective.

Measured on TRN2 via reps=17-vs-9 back-to-back NEFF differencing at
150-iter pipelined dispatch (the only profiling on this axon build):
~176 us/body (run-to-run spread 161-202); collective-free variant
154 us; TimelineSim cost model 138 us.  v1 (fp32r, 4-rank AGs)
measured 359 us by the same method.  Max rel err 2.017e-3.
"""

import sys

sys.path.insert(0, "/opt/trn_rl_repo")

import numpy as np
import ml_dtypes

import concourse.bass as bass
import concourse.mybir as mybir
from concourse import tile
from concourse.bass_utils import run_bass_kernel_spmd

F32 = mybir.dt.float32
BF16 = mybir.dt.bfloat16
I32 = mybir.dt.int32

B, S, D = 2, 2048, 1024
NEG = 10000.0
SCALE = 1.0 / 32.0  # 1/sqrt(D)

# ---------------------------------------------------------------------------
# Post-scheduling pass: split multi-wait instructions into NOP chains.
# The pinned walrus codegen accepts only one embedded sync-wait per
# instruction on most engine instruction formats; Tile's semaphore
# assignment freely emits several.  Rewrite each instruction with k>1
# waits into (k-1) same-engine NoOps carrying one wait each, inserted
# immediately before it (same engine program order => semantics kept).
# ---------------------------------------------------------------------------
_WSPLIT_CTR = [0]


def _split_waits(nc, max_waits=1):
    n = 0
    for f in nc.m.functions:
        for blk in f.blocks:
            out = []
            for inst in blk.instructions:
                si = inst.sync_info
                if si is not None and len(si.on_wait) > max_waits:
                    waits = list(si.on_wait)
                    for w in waits[:-max_waits]:
                        _WSPLIT_CTR[0] += 1
                        nop = mybir.InstNoOp(name=f"WSPLIT-{_WSPLIT_CTR[0]}")
                        nop.engine = inst.engine
                        nop.sync_info = mybir.SyncInfo(on_wait=[w], on_update=[])
                        out.append(nop)
                    inst.sync_info = mybir.SyncInfo(
                        on_wait=waits[-max_waits:], on_update=list(si.on_update)
                    )
                    n += 1
                out.append(inst)
            blk.instructions = out
    return n


# ---------------------------------------------------------------------------
# Kernel program (identical on all 8 cores)
# ---------------------------------------------------------------------------


def _build_program(timing=False, reps=1, no_cc=False):
    """timing=True builds a single-core variant (no collective; gathered
    K^T/V reads redirected to the local bounce buffer) with identical
    instruction mix/volume, for TimelineSim cost-model analysis.
    reps>1 emits the whole kernel body that many times back-to-back
    (benchmarking: amortizes the per-dispatch overhead)."""
    nc = bass.Bass(
        "TRN2", target_bir_lowering=False, debug=False,
        num_devices=1 if timing else 8,
    )

    x_kvT = nc.dram_tensor("x_kvt", [D, 512], BF16, kind="ExternalInput")
    x_qT = nc.dram_tensor("x_qt", [D, 512], BF16, kind="ExternalInput")
    wq_in = nc.dram_tensor("wq", [D, D], BF16, kind="ExternalInput")
    wk_in = nc.dram_tensor("wk", [D, D], BF16, kind="ExternalInput")
    wv_in = nc.dram_tensor("wv", [D, D], BF16, kind="ExternalInput")
    wo_in = nc.dram_tensor("wo", [D, D], BF16, kind="ExternalInput")
    mask_in = nc.dram_tensor("mask", [128, 512], F32, kind="ExternalInput")
    ident_in = nc.dram_tensor("ident", [128, 128], F32, kind="ExternalInput")
    gidx_in = nc.dram_tensor("gidx", [128, 8], I32, kind="ExternalInput")
    y_out = nc.dram_tensor("y_out", [512, D], F32, kind="ExternalOutput")

    with tile.TileContext(nc) as tc:
        for _rep in range(reps):
            _emit(nc, tc, x_kvT, x_qT, wq_in, wk_in, wv_in, wo_in, mask_in,
                  ident_in, gidx_in, y_out, timing or no_cc)

    _split_waits(nc)
    return nc


SECTIONS = []


def _emit(nc, tc, x_kvT, x_qT, wq_in, wk_in, wv_in, wo_in, mask_in, ident_in,
          gidx_in, y_out, timing):
    del SECTIONS[:]

    def mark(label):
        try:
            SECTIONS.append((nc.next_id(), label))
        except Exception:
            pass

    Relu = mybir.ActivationFunctionType.Relu
    Exp = mybir.ActivationFunctionType.Exp
    AX = mybir.AxisListType.X

    pools = []

    def pool(name, bufs, space="SBUF"):
        p = tc.alloc_tile_pool(name=name, bufs=bufs, space=space)
        pools.append(p)
        return p

    # ----- long-lived pools -----
    const_p = pool("const", 1)
    qt_p = pool("qt", 1)
    wo_p = pool("wo", 1)
    e_p = pool("e", 1)
    pt_p = pool("pt", 2)
    y_p = pool("y", 1)
    yt_p = pool("yt", 1)
    out_p = pool("out", 1)
    st_p = pool("st", 2)
    res_p = pool("res", 1)
    dram_p = pool("dram", 1, space="DRAM")

    # const tiles (DMAs emitted later, after the weight streams, so they
    # never delay the first projection matmuls on the sync ring)
    ident_t = const_p.tile([128, 128], F32, tag="ident")
    mask_t = const_p.tile([128, 512], F32, tag="mask")
    gidx_t = const_p.tile([128, 8], I32, tag="gidx")

    qt_t = qt_p.tile([128, 4096], BF16, tag="qt")   # [dout d-tile, 512 q-tok]
    wo_t = wo_p.tile([128, 8192], BF16, tag="wo")   # [din d-tile, 1024 dout]

    # gathered K^T / V residency: all 4 chunks (incl. own) come back from
    # the gathered buffer via indirect DMA -- the own-chunk's gather row
    # position is per-core data, so the program stays core-agnostic.
    kt_res = [res_p.tile([128, 4096], BF16, tag=f"kt{g}", name=f"kt_res{g}")
              for g in range(4)]
    v_res = [res_p.tile([128, 4096], BF16, tag=f"v{g}", name=f"v_res{g}")
             for g in range(4)]

    # collective bounce buffer (DRAM pool tile -> Tile tracks deps).
    # ONE fused K^T+V shard per core: [256 rows, 4096 bf16] (8 KiB rows;
    # rows 0-127 = kt_own, 128-255 = v_own), so the exchange is a single
    # 8-core AllGather (one barrier, one ncfw op) and a gathered chunk
    # reload is ONE per-partition row-gather DMA.
    kv_in = dram_p.tile([256, 4096], BF16, tag="kv_in")
    if not timing:
        kv_gath = dram_p.tile([2048, 4096], BF16, tag="kv_gath",
                              addr_space="Shared")
    else:
        kv_gath = kv_in

    def load_chunk(dst, src, col):
        nc.gpsimd.indirect_dma_start(
            out=dst[:, :],
            out_offset=None,
            in_=src[:, :],
            in_offset=bass.IndirectOffsetOnAxis(ap=gidx_t[:, col:col + 1],
                                                axis=0),
        )

    # =====================================================================
    # Projections: K^T own -> V own -> fused AllGather -> Q^T + Wo
    # =====================================================================
    with tc.tile_pool(name="pP", bufs=1) as pp, \
         tc.tile_pool(name="wstream", bufs=8) as wsp, \
         tc.tile_pool(name="ps_pmm", bufs=8, space="PSUM") as ps_mm:

        xt_kv = pp.tile([128, 4096], BF16, tag="xt_kv")
        xt_q = pp.tile([128, 4096], BF16, tag="xt_q")
        kt_own = pp.tile([128, 4096], BF16, tag="kt_own")
        v_own = pp.tile([128, 4096], BF16, tag="v_own")

        # first-MM-critical DMAs lead the sync ring: x^T d-slice 0, then
        # the first wk slice arrives inside the K-proj loop below.
        mark("xt_kv")
        xkv3 = xt_kv.rearrange("p (d b) -> p d b", b=512)
        nc.sync.dma_start(xkv3[:, 0, :], x_kvT.ap()[0:128, :])

        # ---- K^T own: kt_own[p, 512m+tok] = relu(K^T)[128m+p, tok]
        mark("kT")
        for half in range(2):
            mms = [ps_mm.tile([128, 512], F32, tag="mm", name=f"mmk{half}_{i}")
                   for i in range(4)]
            for d in range(8):
                wk_d = wsp.tile([128, 512], BF16, tag="w", name=f"wk{half}{d}")
                nc.sync.dma_start(
                    wk_d[:], wk_in.ap()[128 * d:128 * (d + 1),
                                        512 * half:512 * (half + 1)])
                if half == 0 and d < 7:
                    nc.sync.dma_start(xkv3[:, d + 1, :],
                                      x_kvT.ap()[128 * (d + 1):128 * (d + 2), :])
                for mi in range(4):
                    nc.tensor.matmul(
                        mms[mi][:],
                        wk_d[:, 128 * mi:128 * (mi + 1)],
                        xt_kv[:, 512 * d:512 * (d + 1)],
                        start=(d == 0), stop=(d == 7),
                    )
            for mi in range(4):
                m = 4 * half + mi
                nc.scalar.activation(kt_own[:, 512 * m:512 * (m + 1)],
                                     mms[mi][:], Relu)
        nc.scalar.dma_start(kv_in[0:128, :], kt_own[:, :])

        # ---- V own: v_own[p, 1024t+dv] = relu(V)[128t+p, dv]
        mark("V")
        for h in range(2):
            mms = [ps_mm.tile([128, 512], F32, tag="mm", name=f"mmv{h}_{i}")
                   for i in range(4)]
            for d in range(8):
                wv_d = wsp.tile([128, 512], BF16, tag="w", name=f"wv{h}{d}")
                nc.sync.dma_start(
                    wv_d[:], wv_in.ap()[128 * d:128 * (d + 1),
                                        512 * h:512 * (h + 1)])
                for t in range(4):
                    nc.tensor.matmul(
                        mms[t][:],
                        xt_kv[:, 512 * d + 128 * t:512 * d + 128 * (t + 1)],
                        wv_d[:],
                        start=(d == 0), stop=(d == 7),
                    )
            for t in range(4):
                nc.scalar.activation(
                    v_own[:, 1024 * t + 512 * h:1024 * t + 512 * (h + 1)],
                    mms[t][:], Relu)
        nc.scalar.dma_start(kv_in[128:256, :], v_own[:, :])

        mark("consts")
        nc.sync.dma_start(gidx_t[:], gidx_in.ap())
        nc.sync.dma_start(ident_t[:], ident_in.ap())
        nc.sync.dma_start(mask_t[:], mask_in.ap())

        mark("cc_kv")
        if not timing:
            nc.gpsimd.collective_compute(
                "AllGather",
                mybir.AluOpType.bypass,
                replica_groups=[[0, 1, 2, 3, 4, 5, 6, 7]],
                ins=[kv_in[:, :]],
                outs=[kv_gath[:, :]],
            )

        mark("xt_q")
        xq3 = xt_q.rearrange("p (d b) -> p d b", b=512)
        for d in range(8):
            nc.sync.dma_start(xq3[:, d, :],
                              x_qT.ap()[128 * d:128 * (d + 1), :])

        # ---- Q^T own (scaled by 1/sqrt(D))
        mark("qT")
        for half in range(2):
            mms = [ps_mm.tile([128, 512], F32, tag="mm", name=f"mmq{half}_{i}")
                   for i in range(4)]
            for d in range(8):
                wq_d = wsp.tile([128, 512], BF16, tag="w", name=f"wq{half}{d}")
                nc.sync.dma_start(
                    wq_d[:], wq_in.ap()[128 * d:128 * (d + 1),
                                        512 * half:512 * (half + 1)])
                for mi in range(4):
                    nc.tensor.matmul(
                        mms[mi][:],
                        wq_d[:, 128 * mi:128 * (mi + 1)],
                        xt_q[:, 512 * d:512 * (d + 1)],
                        start=(d == 0), stop=(d == 7),
                    )
            for mi in range(4):
                m = 4 * half + mi
                nc.scalar.activation(qt_t[:, 512 * m:512 * (m + 1)],
                                     mms[mi][:], Relu, scale=SCALE)

        mark("woload")
        for d in range(8):
            nc.sync.dma_start(wo_t[:, 1024 * d:1024 * (d + 1)],
                              wo_in.ap()[128 * d:128 * (d + 1), :])

        # gathered chunk loads (gpsimd ring, behind the AG trigger), in
        # consumption order: block i's scores need kt_i early; v_i only
        # after block i's softmax.
        mark("kvload")
        for kind, g in [("kt", 0), ("v", 0), ("kt", 1), ("kt", 2),
                        ("v", 1), ("kt", 3), ("v", 2), ("v", 3)]:
            if kind == "kt":
                load_chunk(kt_res[g], kv_gath, g)
            else:
                load_chunk(v_res[g], kv_gath, 4 + g)

    # =====================================================================
    # Attention + output projection, software-pipelined across blocks
    # =====================================================================
    with tc.tile_pool(name="ps_cmm", bufs=2, space="PSUM") as ps_mm, \
         tc.tile_pool(name="ps_ctr", bufs=2, space="PSUM") as ps_tr, \
         tc.tile_pool(name="ps_y", bufs=1, space="PSUM") as ps_y, \
         tc.tile_pool(name="ps_yt", bufs=1, space="PSUM") as ps_yt:

        st = {}

        def emit_e(i):
            mark(f"e{i}")
            if i == 0:
                # the [128,1024] output-staging slot is idle until tail0 and
                # block0's scores die before tail0 -- reuse it so block1's
                # copies never wait on block0's transposes.
                e_t = out_p.tile([128, 1024], F32, tag="osb", name="e0")[:, 0:512]
            else:
                e_t = e_p.tile([128, 512 * (i + 1)], F32,
                               tag=("eA" if i < 2 else "eB"), name=f"e{i}")
            st[i] = {"e": e_t}
            negmax = None
            for g in range(i + 1):
                mm = ps_mm.tile([128, 512], F32, tag="mm", name=f"mme{i}{g}")
                ktg = kt_res[g].rearrange("p (d b) -> p d b", b=512)
                for d in range(8):
                    nc.tensor.matmul(
                        mm[:],
                        qt_t[:, 512 * d + 128 * i:512 * d + 128 * (i + 1)],
                        ktg[:, d, :],
                        start=(d == 0), stop=(d == 7),
                    )
                if g == i:
                    nc.vector.tensor_add(e_t[:, 512 * g:512 * (g + 1)],
                                         mm[:], mask_t[:])
                else:
                    nc.vector.tensor_copy(e_t[:, 512 * g:512 * (g + 1)], mm[:])
                # incremental per-chunk negated max, hidden behind the next
                # chunk's score matmuls: negmax = min_g(-max(chunk_g))
                nm_g = st_p.tile([128, 1], F32, tag="nmg", name=f"nm{i}_{g}")
                nc.vector.reduce_max(nm_g[:], e_t[:, 512 * g:512 * (g + 1)],
                                     axis=AX, negate=True)
                if negmax is None:
                    negmax = nm_g
                else:
                    acc = st_p.tile([128, 1], F32, tag="nmacc",
                                    name=f"nma{i}_{g}")
                    nc.vector.tensor_tensor(acc[:], negmax[:], nm_g[:],
                                            op=mybir.AluOpType.min)
                    negmax = acc
            st[i]["negmax"] = negmax

        def emit_softmax(i):
            mark(f"sm{i}")
            e_t = st[i]["e"]
            W = 512 * (i + 1)
            rowsum = st_p.tile([128, 1], F32, tag="rowsum", name=f"rs{i}")
            nc.scalar.activation(e_t[:, 0:W], e_t[:, 0:W], Exp,
                                 bias=st[i]["negmax"][:], scale=1.0,
                                 accum_out=rowsum[:])
            rinv = st_p.tile([128, 1], F32, tag="rinv", name=f"ri{i}")
            nc.vector.reciprocal(rinv[:], rowsum[:])
            st[i]["rinv"] = rinv

        def emit_trav(i):
            mark(f"av{i}")
            e_t = st[i]["e"]
            yps = ps_y.tile([128, 1024], F32, tag="yacc", name=f"y{i}")
            st[i]["yps"] = yps
            for g in range(i + 1):
                trp = ps_tr.tile([128, 512], F32, tag="ctr", name=f"ctr{i}{g}")
                for j in range(4):
                    nc.tensor.transpose(
                        trp[:, 128 * j:128 * (j + 1)],
                        e_t[:, 512 * g + 128 * j:512 * g + 128 * (j + 1)],
                        ident_t[:],
                    )
                pt_t = pt_p.tile([128, 512], BF16, tag="pt", name=f"pt{i}{g}")
                nc.vector.tensor_copy(pt_t[:], trp[:])
                vg = v_res[g].rearrange("p (t b) -> p t b", b=1024)
                for j in range(4):
                    for h in range(2):
                        nc.tensor.matmul(
                            yps[:, 512 * h:512 * (h + 1)],
                            pt_t[:, 128 * j:128 * (j + 1)],
                            vg[:, j, 512 * h:512 * (h + 1)],
                            start=(g == 0 and j == 0),
                            stop=(g == i and j == 3),
                        )

        def emit_tail(i):
            # y stays unnormalized; 1/rowsum is applied as the per-partition
            # scale of the final relu (relu(a*c) = relu(a)*c for c > 0).
            mark(f"tail{i}")
            y_t = y_p.tile([128, 1024], F32, tag="ysb", name=f"ysb{i}")
            nc.vector.tensor_copy(y_t[:, 0:512], st[i]["yps"][:, 0:512])
            nc.vector.tensor_copy(y_t[:, 512:1024], st[i]["yps"][:, 512:1024])
            ytp = ps_yt.tile([128, 1024], F32, tag="ytp", name=f"ytp{i}")
            for d in range(8):
                nc.tensor.transpose(
                    ytp[:, 128 * d:128 * (d + 1)],
                    y_t[:, 128 * d:128 * (d + 1)],
                    ident_t[:],
                )
            yt_t = yt_p.tile([128, 1024], BF16, tag="ytsb", name=f"ytsb{i}")
            nc.vector.tensor_copy(yt_t[:], ytp[:])
            o_t = out_p.tile([128, 1024], F32, tag="osb", name=f"osb{i}")
            for h in range(2):
                mm = ps_mm.tile([128, 512], F32, tag="mm", name=f"mmo{i}{h}")
                for d in range(8):
                    nc.tensor.matmul(
                        mm[:],
                        yt_t[:, 128 * d:128 * (d + 1)],
                        wo_t[:, 1024 * d + 512 * h:1024 * d + 512 * (h + 1)],
                        start=(d == 0), stop=(d == 7),
                    )
                nc.scalar.activation(o_t[:, 512 * h:512 * (h + 1)], mm[:], Relu,
                                     scale=st[i]["rinv"][:])
                # per-half store: the second half's relu overlaps the first
                # half's writeback, shortening the final-block drain.
                nc.scalar.dma_start(
                    y_out.ap()[128 * i:128 * (i + 1), 512 * h:512 * (h + 1)],
                    o_t[:, 512 * h:512 * (h + 1)])

        # pipelined emission: PE fills softmax bubbles with the next
        # block's score matmuls.
        emit_e(0)
        emit_softmax(0)
        emit_e(1)
        emit_trav(0)
        emit_tail(0)
        emit_softmax(1)
        emit_e(2)
        emit_trav(1)
        emit_tail(1)
        emit_softmax(2)
        emit_e(3)
        emit_trav(2)
        emit_tail(2)
        emit_softmax(3)
        emit_trav(3)
        emit_tail(3)

    mark("end")
    for p in reversed(pools):
        p.release()


_PROGRAM_CACHE = {}


def _get_program():
    if "nc" not in _PROGRAM_CACHE:
        _PROGRAM_CACHE["nc"] = _build_program()
    return _PROGRAM_CACHE["nc"]


# ---------------------------------------------------------------------------
# Host-side entry point
# ---------------------------------------------------------------------------


def _bf16(a):
    return np.asarray(np.asarray(a, dtype=np.float32),
                      dtype=ml_dtypes.bfloat16)


def _make_mask(r):
    i = np.arange(128)[:, None]
    j = np.arange(512)[None, :]
    return np.where(j > 128 * r + i, np.float32(-NEG), np.float32(0.0))


def _make_in_maps(x, Wq, Wk, Wv, Wo):
    x = np.asarray(x, dtype=np.float32)
    wq = _bf16(Wq)
    wk = _bf16(Wk)
    wv = _bf16(Wv)
    wo = _bf16(Wo)
    ident = np.eye(128, dtype=np.float32)


    in_maps = []
    for core in range(8):
        b, r = divmod(core, 4)
        xb = x[b]
        x_kvT = np.ascontiguousarray(_bf16(xb[512 * r:512 * (r + 1)]).T)
        chunks = [r, r + 4, r + 8, r + 12]
        x_q = np.concatenate([xb[128 * c:128 * (c + 1)] for c in chunks],
                             axis=0)
        x_qT = np.ascontiguousarray(_bf16(x_q).T)
        # fused-shard gather rows: core c's shard occupies rows
        # [256c, 256c+256) of kv_gath; kt chunk at +0, v chunk at +128.
        p = np.arange(128, dtype=np.int32)
        gidx = np.empty((128, 8), dtype=np.int32)
        for g in range(4):
            gidx[:, g] = 256 * (4 * b + g) + p
            gidx[:, 4 + g] = 256 * (4 * b + g) + 128 + p
        in_maps.append({
            "x_kvt": x_kvT, "x_qt": x_qT,
            "wq": wq, "wk": wk, "wv": wv, "wo": wo,
            "mask": _make_mask(r), "ident": ident, "gidx": gidx,
        })
    return in_maps


def kernel(x, Wq, bq, Wk, bk, Wv, bv, Wo, bo, _bench=None):
    nc = _get_program()
    in_maps = _make_in_maps(x, Wq, Wk, Wv, Wo)

    kwargs = dict(_bench or {})
    res = run_bass_kernel_spmd(nc, in_maps, list(range(8)), **kwargs)

    out = np.empty((B, S, D), dtype=np.float32)
    for core in range(8):
        b, r = divmod(core, 4)
        yo = res.results[core]["y_out"]
        for i, c in enumerate([r, r + 4, r + 8, r + 12]):
            out[b, 128 * c:128 * (c + 1), :] = yo[128 * i:128 * (i + 1), :]
    if _bench is not None:
        kernel.last_result = res
    return out


kernel.last_result = None


# ---------------------------------------------------------------------------
# Benchmarking helper (used by test.py only): runs the kernel repeatedly
# through a persistent jitted PJRT executable with device-resident inputs,
# so per-call wall time approximates dispatch-overhead + HW exec time.
# ---------------------------------------------------------------------------


def make_runner(nc, in_maps, chain=1):
    """chain>1 invokes the NEFF that many times inside ONE jitted XLA
    program, feeding iteration i's outputs as iteration i+1's output
    seed buffers (true data dependence -> no CSE, serialized NEFF
    executions).  Per-call dispatch overhead is paid once, so
    (wall(chain=K) - wall(chain=1)) / (K-1) isolates HW exec time."""
    import jax
    from jax.sharding import Mesh, PartitionSpec, NamedSharding
    from jax.experimental.shard_map import shard_map
    from concourse.bass2jax import (
        _bass_exec_p, install_neuronx_cc_hook, partition_id_tensor,
    )

    install_neuronx_cc_hook()
    n_cores = len(in_maps)
    in_names, out_names, out_avals, zero_outs = [], [], [], []
    pname = nc.partition_id_tensor.name if nc.partition_id_tensor else None
    for alloc in nc.m.functions[0].allocations:
        if not isinstance(alloc, mybir.MemoryLocationSet):
            continue
        name = alloc.memorylocations[0].name
        if alloc.kind == "ExternalInput":
            if name != pname:
                in_names.append(name)
        elif alloc.kind == "ExternalOutput":
            shape = tuple(alloc.tensor_shape)
            dtype = mybir.dt.np(alloc.dtype)
            out_names.append(name)
            out_avals.append(jax.core.ShapedArray(shape, dtype))
            zero_outs.append(np.zeros(shape, dtype))
    n_params = len(in_names)
    all_in = list(in_names) + list(out_names)
    if pname:
        all_in.append(pname)

    def _body(*args):
        ins = list(args[:n_params])
        outs = tuple(args[n_params:])
        for _ in range(chain):
            operands = ins + list(outs)
            if pname is not None:
                operands.append(partition_id_tensor())
            outs = tuple(_bass_exec_p.bind(
                *operands, out_avals=tuple(out_avals), in_names=tuple(all_in),
                out_names=tuple(out_names), lowering_input_output_aliases=(),
                sim_require_finite=True, sim_require_nnan=True, nc=nc))
        return outs

    devices = jax.devices()[:n_cores]
    mesh = Mesh(np.asarray(devices), ("core",))
    specs_in = (PartitionSpec("core"),) * (n_params + len(out_names))
    specs_out = (PartitionSpec("core"),) * len(out_names)
    fn = jax.jit(shard_map(_body, mesh=mesh, in_specs=specs_in,
                           out_specs=specs_out, check_rep=False),
                 keep_unused=True)
    sh = NamedSharding(mesh, PartitionSpec("core"))
    concat_in = [np.concatenate([np.asarray(m[n]) for m in in_maps], axis=0)
                 for n in in_names]
    concat_zero = [np.zeros((n_cores * z.shape[0], *z.shape[1:]), z.dtype)
                   for z in zero_outs]
    dev_in = [jax.device_put(a, sh) for a in concat_in]
    dev_zero = [jax.device_put(a, sh) for a in concat_zero]
    return fn, dev_in, dev_zero, out_names


def bench_hw(inputs, iters=60, trials=3):
    """Pipelined-dispatch wall-clock per call for the real 8-core program
    and for a trivial null program; the difference estimates HW exec time."""
    import time
    import jax

    in_maps = _make_in_maps(inputs["x"], inputs["Wq"], inputs["Wk"],
                            inputs["Wv"], inputs["Wo"])

    def null_program():
        nnc = bass.Bass("TRN2", target_bir_lowering=False, debug=False,
                        num_devices=8)
        xi = nnc.dram_tensor("xn", [128, 128], F32, kind="ExternalInput")
        yo = nnc.dram_tensor("yn", [128, 128], F32, kind="ExternalOutput")
        with tile.TileContext(nnc) as tcc:
            with tcc.tile_pool(name="s", bufs=1) as pl:
                t = pl.tile([128, 128], F32, name="t0")
                nnc.sync.dma_start(t[:], xi.ap())
                nnc.sync.dma_start(yo.ap(), t[:])
        _split_waits(nnc)
        return nnc, [{"xn": np.zeros((128, 128), np.float32)}] * 8

    def measure(fn, di, dz):
        out = fn(*di, *dz)
        jax.block_until_ready(out)
        best = float("inf")
        for _ in range(trials):
            t0 = time.perf_counter()
            outs = [fn(*di, *dz) for _ in range(iters)]
            jax.block_until_ready(outs)
            dt = (time.perf_counter() - t0) / iters
            best = min(best, dt)
        return best

    nnc, null_maps = null_program()
    fn0, di0, dz0, _ = make_runner(nnc, null_maps)
    t_null = measure(fn0, di0, dz0)
    fn1, di1, dz1, _ = make_runner(_get_program(), in_maps)
    t_full = measure(fn1, di1, dz1)
    return t_full, t_null


# revision 30
# speedup vs baseline: 1.9006x; 1.1221x over previous
"""Trainium2 Bass kernel for nn_MultiHeadAttention_32066225832689.

Reference computation (B=2, S=2048, D=1024, fp32):
    q = relu(x @ Wq + bq); k = relu(x @ Wk + bk); v = relu(x @ Wv + bv)
    e = (q @ k^T) / sqrt(D);  e -= 10000 * causal_mask
    attn = softmax(e);  y = relu((attn @ v) @ Wo + bo)
Biases are all zeros by problem spec (fill: zeros) and are ignored.

Sharding over 8 NeuronCores: batch (2) x rank (4).  Rank r of a batch
group owns:
  - K/V projection for token rows [512r, 512r+512) (data-parallel).
    Both tensors are exchanged in ONE fused 8-core AllGather with a
    Shared-address-space output (the fast collective path; Shared
    output needs >4-core groups, and the 4-rank grouped AllGather
    takes the slow fold_n=2 ring: ~77us/AG vs the whole fused
    exchange costing ~60us steady-state here.  Two separate 8-core
    AGs serialize on ncfw and measured 45us slower than one fused).
    The 8-core gather mixes both batch groups' chunks, so each core
    picks its batch's four chunks out of the gathered buffer with
    per-partition row-gather indirect DMAs whose row indices are
    host-provided per-core data -- the SPMD program stays identical
    on all cores.  (Static conditional DMAs gated on a host flag via
    values_load would avoid the SWDGE path, but the pinned walrus
    cannot encode the bounds-check register ISA: "ISA wrong length".)
  - Query chunks {r, r+4, r+8, r+12} (128 rows each).  Chunk c needs
    key chunks 0..c//4, so every rank processes blocks with 1,2,3,4
    key chunks of 512 -- a balanced, rank-uniform causal workload.

The whole data plane runs in bfloat16 (weights, x^T, K^T, V, Q^T, P):
same 1 cycle/row PE rate as fp32r but half the DMA/collective bytes
and SBUF footprint.  Scores/softmax/output accumulation stay fp32.
Numpy modeling of this rounding placement predicts rel err ~4e-3 and
hardware measures 2.0e-3 against the fp32 reference (budget 2e-2).
fp8e4m3 K/V was evaluated numerically at 3.7e-2 -- over budget.

x^T is pre-transposed on the host (free), removing v1's on-device PE
transpose stage.  The row max is accumulated incrementally per score
chunk (hidden behind the next chunk's matmuls) and 1/rowsum is folded
into the final relu's per-partition scale.  Weights/x stream on the
sync HWDGE ring (first x^T slice and first wk slice lead; consts
trail), bounce writes + output stores on the scalar ring, collective +
gathered-chunk loads on the gpsimd ring -- no input stream ever waits
on the collective.

Measured on TRN2 via reps=17-vs-9 back-to-back NEFF differencing at
150-iter pipelined dispatch (the only profiling on this axon build):
median 189 us/body over 6 rounds (spread 161-213); collective-free
variant 154 us; TimelineSim cost model 138 us.  v1 (fp32r, 4-rank
AGs) measured 359 us by the same method.  Max rel err 2.017e-3.
"""

import sys

sys.path.insert(0, "/opt/trn_rl_repo")

import numpy as np
import ml_dtypes

import concourse.bass as bass
import concourse.mybir as mybir
from concourse import tile
from concourse.bass_utils import run_bass_kernel_spmd

F32 = mybir.dt.float32
BF16 = mybir.dt.bfloat16
I32 = mybir.dt.int32

B, S, D = 2, 2048, 1024
NEG = 10000.0
SCALE = 1.0 / 32.0  # 1/sqrt(D)

# ---------------------------------------------------------------------------
# Post-scheduling pass: split multi-wait instructions into NOP chains.
# The pinned walrus codegen accepts only one embedded sync-wait per
# instruction on most engine instruction formats; Tile's semaphore
# assignment freely emits several.  Rewrite each instruction with k>1
# waits into (k-1) same-engine NoOps carrying one wait each, inserted
# immediately before it (same engine program order => semantics kept).
# ---------------------------------------------------------------------------
_WSPLIT_CTR = [0]


def _split_waits(nc, max_waits=1):
    n = 0
    for f in nc.m.functions:
        for blk in f.blocks:
            out = []
            for inst in blk.instructions:
                si = inst.sync_info
                if si is not None and len(si.on_wait) > max_waits:
                    waits = list(si.on_wait)
                    for w in waits[:-max_waits]:
                        _WSPLIT_CTR[0] += 1
                        nop = mybir.InstNoOp(name=f"WSPLIT-{_WSPLIT_CTR[0]}")
                        nop.engine = inst.engine
                        nop.sync_info = mybir.SyncInfo(on_wait=[w], on_update=[])
                        out.append(nop)
                    inst.sync_info = mybir.SyncInfo(
                        on_wait=waits[-max_waits:], on_update=list(si.on_update)
                    )
                    n += 1
                out.append(inst)
            blk.instructions = out
    return n


# ---------------------------------------------------------------------------
# Kernel program (identical on all 8 cores)
# ---------------------------------------------------------------------------


def _build_program(timing=False, reps=1, no_cc=False):
    """timing=True builds a single-core variant (no collective; gathered
    K^T/V reads redirected to the local bounce buffer) with identical
    instruction mix/volume, for TimelineSim cost-model analysis.
    reps>1 emits the whole kernel body that many times back-to-back
    (benchmarking: amortizes the per-dispatch overhead)."""
    nc = bass.Bass(
        "TRN2", target_bir_lowering=False, debug=False,
        num_devices=1 if timing else 8,
    )

    x_kvT = nc.dram_tensor("x_kvt", [D, 512], BF16, kind="ExternalInput")
    x_qT = nc.dram_tensor("x_qt", [D, 512], BF16, kind="ExternalInput")
    wq_in = nc.dram_tensor("wq", [D, D], BF16, kind="ExternalInput")
    wk_in = nc.dram_tensor("wk", [D, D], BF16, kind="ExternalInput")
    wv_in = nc.dram_tensor("wv", [D, D], BF16, kind="ExternalInput")
    wo_in = nc.dram_tensor("wo", [D, D], BF16, kind="ExternalInput")
    mask_in = nc.dram_tensor("mask", [128, 512], F32, kind="ExternalInput")
    ident_in = nc.dram_tensor("ident", [128, 128], F32, kind="ExternalInput")
    gidx_in = nc.dram_tensor("gidx", [128, 8], I32, kind="ExternalInput")
    y_out = nc.dram_tensor("y_out", [512, D], F32, kind="ExternalOutput")

    with tile.TileContext(nc) as tc:
        for _rep in range(reps):
            _emit(nc, tc, x_kvT, x_qT, wq_in, wk_in, wv_in, wo_in, mask_in,
                  ident_in, gidx_in, y_out, timing or no_cc)

    _split_waits(nc)
    return nc


SECTIONS = []


def _emit(nc, tc, x_kvT, x_qT, wq_in, wk_in, wv_in, wo_in, mask_in, ident_in,
          gidx_in, y_out, timing):
    del SECTIONS[:]

    def mark(label):
        try:
            SECTIONS.append((nc.next_id(), label))
        except Exception:
            pass

    Relu = mybir.ActivationFunctionType.Relu
    Exp = mybir.ActivationFunctionType.Exp
    AX = mybir.AxisListType.X

    pools = []

    def pool(name, bufs, space="SBUF"):
        p = tc.alloc_tile_pool(name=name, bufs=bufs, space=space)
        pools.append(p)
        return p

    # ----- long-lived pools -----
    const_p = pool("const", 1)
    qt_p = pool("qt", 1)
    wo_p = pool("wo", 1)
    e_p = pool("e", 1)
    pt_p = pool("pt", 2)
    y_p = pool("y", 1)
    yt_p = pool("yt", 1)
    out_p = pool("out", 1)
    st_p = pool("st", 2)
    res_p = pool("res", 1)
    dram_p = pool("dram", 1, space="DRAM")

    # const tiles (DMAs emitted later, after the weight streams, so they
    # never delay the first projection matmuls on the sync ring)
    ident_t = const_p.tile([128, 128], F32, tag="ident")
    mask_t = const_p.tile([128, 512], F32, tag="mask")
    gidx_t = const_p.tile([128, 8], I32, tag="gidx")

    qt_t = qt_p.tile([128, 4096], BF16, tag="qt")   # [dout d-tile, 512 q-tok]
    wo_t = wo_p.tile([128, 8192], BF16, tag="wo")   # [din d-tile, 1024 dout]

    # gathered K^T / V residency: all 4 chunks (incl. own) come back from
    # the gathered buffer via indirect DMA -- the own-chunk's gather row
    # position is per-core data, so the program stays core-agnostic.
    kt_res = [res_p.tile([128, 4096], BF16, tag=f"kt{g}", name=f"kt_res{g}")
              for g in range(4)]
    v_res = [res_p.tile([128, 4096], BF16, tag=f"v{g}", name=f"v_res{g}")
             for g in range(4)]

    # collective bounce buffer (DRAM pool tile -> Tile tracks deps).
    # ONE fused K^T+V shard per core: [256 rows, 4096 bf16] (8 KiB rows;
    # rows 0-127 = kt_own, 128-255 = v_own), so the exchange is a single
    # 8-core AllGather (one barrier, one ncfw op) and a gathered chunk
    # reload is ONE per-partition row-gather DMA.
    kv_in = dram_p.tile([256, 4096], BF16, tag="kv_in")
    if not timing:
        kv_gath = dram_p.tile([2048, 4096], BF16, tag="kv_gath",
                              addr_space="Shared")
    else:
        kv_gath = kv_in

    def load_chunk(dst, src, col):
        nc.gpsimd.indirect_dma_start(
            out=dst[:, :],
            out_offset=None,
            in_=src[:, :],
            in_offset=bass.IndirectOffsetOnAxis(ap=gidx_t[:, col:col + 1],
                                                axis=0),
        )

    # =====================================================================
    # Projections: K^T own -> V own -> fused AllGather -> Q^T + Wo
    # =====================================================================
    with tc.tile_pool(name="pP", bufs=1) as pp, \
         tc.tile_pool(name="wstream", bufs=8) as wsp, \
         tc.tile_pool(name="ps_pmm", bufs=8, space="PSUM") as ps_mm:

        xt_kv = pp.tile([128, 4096], BF16, tag="xt_kv")
        xt_q = pp.tile([128, 4096], BF16, tag="xt_q")
        kt_own = pp.tile([128, 4096], BF16, tag="kt_own")
        v_own = pp.tile([128, 4096], BF16, tag="v_own")

        # first-MM-critical DMAs lead the sync ring: x^T d-slice 0, then
        # the first wk slice arrives inside the K-proj loop below.
        mark("xt_kv")
        xkv3 = xt_kv.rearrange("p (d b) -> p d b", b=512)
        nc.sync.dma_start(xkv3[:, 0, :], x_kvT.ap()[0:128, :])

        # ---- K^T own: kt_own[p, 512m+tok] = relu(K^T)[128m+p, tok]
        mark("kT")
        for half in range(2):
            mms = [ps_mm.tile([128, 512], F32, tag="mm", name=f"mmk{half}_{i}")
                   for i in range(4)]
            for d in range(8):
                wk_d = wsp.tile([128, 512], BF16, tag="w", name=f"wk{half}{d}")
                nc.sync.dma_start(
                    wk_d[:], wk_in.ap()[128 * d:128 * (d + 1),
                                        512 * half:512 * (half + 1)])
                if half == 0 and d < 7:
                    nc.sync.dma_start(xkv3[:, d + 1, :],
                                      x_kvT.ap()[128 * (d + 1):128 * (d + 2), :])
                for mi in range(4):
                    nc.tensor.matmul(
                        mms[mi][:],
                        wk_d[:, 128 * mi:128 * (mi + 1)],
                        xt_kv[:, 512 * d:512 * (d + 1)],
                        start=(d == 0), stop=(d == 7),
                    )
            for mi in range(4):
                m = 4 * half + mi
                nc.scalar.activation(kt_own[:, 512 * m:512 * (m + 1)],
                                     mms[mi][:], Relu)
        nc.scalar.dma_start(kv_in[0:128, :], kt_own[:, :])

        # ---- V own: v_own[p, 1024t+dv] = relu(V)[128t+p, dv]
        mark("V")
        for h in range(2):
            mms = [ps_mm.tile([128, 512], F32, tag="mm", name=f"mmv{h}_{i}")
                   for i in range(4)]
            for d in range(8):
                wv_d = wsp.tile([128, 512], BF16, tag="w", name=f"wv{h}{d}")
                nc.sync.dma_start(
                    wv_d[:], wv_in.ap()[128 * d:128 * (d + 1),
                                        512 * h:512 * (h + 1)])
                for t in range(4):
                    nc.tensor.matmul(
                        mms[t][:],
                        xt_kv[:, 512 * d + 128 * t:512 * d + 128 * (t + 1)],
                        wv_d[:],
                        start=(d == 0), stop=(d == 7),
                    )
            for t in range(4):
                nc.scalar.activation(
                    v_own[:, 1024 * t + 512 * h:1024 * t + 512 * (h + 1)],
                    mms[t][:], Relu)
        nc.scalar.dma_start(kv_in[128:256, :], v_own[:, :])

        mark("consts")
        nc.sync.dma_start(gidx_t[:], gidx_in.ap())
        nc.sync.dma_start(ident_t[:], ident_in.ap())
        nc.sync.dma_start(mask_t[:], mask_in.ap())

        mark("cc_kv")
        if not timing:
            nc.gpsimd.collective_compute(
                "AllGather",
                mybir.AluOpType.bypass,
                replica_groups=[[0, 1, 2, 3, 4, 5, 6, 7]],
                ins=[kv_in[:, :]],
                outs=[kv_gath[:, :]],
            )

        mark("xt_q")
        xq3 = xt_q.rearrange("p (d b) -> p d b", b=512)
        for d in range(8):
            nc.sync.dma_start(xq3[:, d, :],
                              x_qT.ap()[128 * d:128 * (d + 1), :])

        # ---- Q^T own (scaled by 1/sqrt(D))
        mark("qT")
        for half in range(2):
            mms = [ps_mm.tile([128, 512], F32, tag="mm", name=f"mmq{half}_{i}")
                   for i in range(4)]
            for d in range(8):
                wq_d = wsp.tile([128, 512], BF16, tag="w", name=f"wq{half}{d}")
                nc.sync.dma_start(
                    wq_d[:], wq_in.ap()[128 * d:128 * (d + 1),
                                        512 * half:512 * (half + 1)])
                for mi in range(4):
                    nc.tensor.matmul(
                        mms[mi][:],
                        wq_d[:, 128 * mi:128 * (mi + 1)],
                        xt_q[:, 512 * d:512 * (d + 1)],
                        start=(d == 0), stop=(d == 7),
                    )
            for mi in range(4):
                m = 4 * half + mi
                nc.scalar.activation(qt_t[:, 512 * m:512 * (m + 1)],
                                     mms[mi][:], Relu, scale=SCALE)

        mark("woload")
        for d in range(8):
            nc.sync.dma_start(wo_t[:, 1024 * d:1024 * (d + 1)],
                              wo_in.ap()[128 * d:128 * (d + 1), :])

        # gathered chunk loads (gpsimd ring, behind the AG trigger), in
        # consumption order: block i's scores need kt_i early; v_i only
        # after block i's softmax.
        mark("kvload")
        for kind, g in [("kt", 0), ("v", 0), ("kt", 1), ("kt", 2),
                        ("v", 1), ("kt", 3), ("v", 2), ("v", 3)]:
            if kind == "kt":
                load_chunk(kt_res[g], kv_gath, g)
            else:
                load_chunk(v_res[g], kv_gath, 4 + g)

    # =====================================================================
    # Attention + output projection, software-pipelined across blocks
    # =====================================================================
    with tc.tile_pool(name="ps_cmm", bufs=2, space="PSUM") as ps_mm, \
         tc.tile_pool(name="ps_ctr", bufs=2, space="PSUM") as ps_tr, \
         tc.tile_pool(name="ps_y", bufs=1, space="PSUM") as ps_y, \
         tc.tile_pool(name="ps_yt", bufs=1, space="PSUM") as ps_yt:

        st = {}

        def emit_e(i):
            mark(f"e{i}")
            if i == 0:
                # the [128,1024] output-staging slot is idle until tail0 and
                # block0's scores die before tail0 -- reuse it so block1's
                # copies never wait on block0's transposes.
                e_t = out_p.tile([128, 1024], F32, tag="osb", name="e0")[:, 0:512]
            else:
                e_t = e_p.tile([128, 512 * (i + 1)], F32,
                               tag=("eA" if i < 2 else "eB"), name=f"e{i}")
            st[i] = {"e": e_t}
            negmax = None
            for g in range(i + 1):
                mm = ps_mm.tile([128, 512], F32, tag="mm", name=f"mme{i}{g}")
                ktg = kt_res[g].rearrange("p (d b) -> p d b", b=512)
                for d in range(8):
                    nc.tensor.matmul(
                        mm[:],
                        qt_t[:, 512 * d + 128 * i:512 * d + 128 * (i + 1)],
                        ktg[:, d, :],
                        start=(d == 0), stop=(d == 7),
                    )
                if g == i:
                    nc.vector.tensor_add(e_t[:, 512 * g:512 * (g + 1)],
                                         mm[:], mask_t[:])
                else:
                    nc.vector.tensor_copy(e_t[:, 512 * g:512 * (g + 1)], mm[:])
                # incremental per-chunk negated max, hidden behind the next
                # chunk's score matmuls: negmax = min_g(-max(chunk_g))
                nm_g = st_p.tile([128, 1], F32, tag="nmg", name=f"nm{i}_{g}")
                nc.vector.reduce_max(nm_g[:], e_t[:, 512 * g:512 * (g + 1)],
                                     axis=AX, negate=True)
                if negmax is None:
                    negmax = nm_g
                else:
                    acc = st_p.tile([128, 1], F32, tag="nmacc",
                                    name=f"nma{i}_{g}")
                    nc.vector.tensor_tensor(acc[:], negmax[:], nm_g[:],
                                            op=mybir.AluOpType.min)
                    negmax = acc
            st[i]["negmax"] = negmax

        def emit_softmax(i):
            mark(f"sm{i}")
            e_t = st[i]["e"]
            W = 512 * (i + 1)
            rowsum = st_p.tile([128, 1], F32, tag="rowsum", name=f"rs{i}")
            nc.scalar.activation(e_t[:, 0:W], e_t[:, 0:W], Exp,
                                 bias=st[i]["negmax"][:], scale=1.0,
                                 accum_out=rowsum[:])
            rinv = st_p.tile([128, 1], F32, tag="rinv", name=f"ri{i}")
            nc.vector.reciprocal(rinv[:], rowsum[:])
            st[i]["rinv"] = rinv

        def emit_trav(i):
            mark(f"av{i}")
            e_t = st[i]["e"]
            yps = ps_y.tile([128, 1024], F32, tag="yacc", name=f"y{i}")
            st[i]["yps"] = yps
            for g in range(i + 1):
                trp = ps_tr.tile([128, 512], F32, tag="ctr", name=f"ctr{i}{g}")
                for j in range(4):
                    nc.tensor.transpose(
                        trp[:, 128 * j:128 * (j + 1)],
                        e_t[:, 512 * g + 128 * j:512 * g + 128 * (j + 1)],
                        ident_t[:],
                    )
                pt_t = pt_p.tile([128, 512], BF16, tag="pt", name=f"pt{i}{g}")
                nc.vector.tensor_copy(pt_t[:], trp[:])
                vg = v_res[g].rearrange("p (t b) -> p t b", b=1024)
                for j in range(4):
                    for h in range(2):
                        nc.tensor.matmul(
                            yps[:, 512 * h:512 * (h + 1)],
                            pt_t[:, 128 * j:128 * (j + 1)],
                            vg[:, j, 512 * h:512 * (h + 1)],
                            start=(g == 0 and j == 0),
                            stop=(g == i and j == 3),
                        )

        def emit_tail(i):
            # y stays unnormalized; 1/rowsum is applied as the per-partition
            # scale of the final relu (relu(a*c) = relu(a)*c for c > 0).
            mark(f"tail{i}")
            y_t = y_p.tile([128, 1024], F32, tag="ysb", name=f"ysb{i}")
            nc.vector.tensor_copy(y_t[:, 0:512], st[i]["yps"][:, 0:512])
            nc.vector.tensor_copy(y_t[:, 512:1024], st[i]["yps"][:, 512:1024])
            ytp = ps_yt.tile([128, 1024], F32, tag="ytp", name=f"ytp{i}")
            for d in range(8):
                nc.tensor.transpose(
                    ytp[:, 128 * d:128 * (d + 1)],
                    y_t[:, 128 * d:128 * (d + 1)],
                    ident_t[:],
                )
            yt_t = yt_p.tile([128, 1024], BF16, tag="ytsb", name=f"ytsb{i}")
            nc.vector.tensor_copy(yt_t[:], ytp[:])
            o_t = out_p.tile([128, 1024], F32, tag="osb", name=f"osb{i}")
            for h in range(2):
                mm = ps_mm.tile([128, 512], F32, tag="mm", name=f"mmo{i}{h}")
                for d in range(8):
                    nc.tensor.matmul(
                        mm[:],
                        yt_t[:, 128 * d:128 * (d + 1)],
                        wo_t[:, 1024 * d + 512 * h:1024 * d + 512 * (h + 1)],
                        start=(d == 0), stop=(d == 7),
                    )
                nc.scalar.activation(o_t[:, 512 * h:512 * (h + 1)], mm[:], Relu,
                                     scale=st[i]["rinv"][:])
                # per-half store: the second half's relu overlaps the first
                # half's writeback, shortening the final-block drain.
                nc.scalar.dma_start(
                    y_out.ap()[128 * i:128 * (i + 1), 512 * h:512 * (h + 1)],
                    o_t[:, 512 * h:512 * (h + 1)])

        # pipelined emission: PE fills softmax bubbles with the next
        # block's score matmuls.
        emit_e(0)
        emit_softmax(0)
        emit_e(1)
        emit_trav(0)
        emit_tail(0)
        emit_softmax(1)
        emit_e(2)
        emit_trav(1)
        emit_tail(1)
        emit_softmax(2)
        emit_e(3)
        emit_trav(2)
        emit_tail(2)
        emit_softmax(3)
        emit_trav(3)
        emit_tail(3)

    mark("end")
    for p in reversed(pools):
        p.release()


_PROGRAM_CACHE = {}


def _get_program():
    if "nc" not in _PROGRAM_CACHE:
        _PROGRAM_CACHE["nc"] = _build_program()
    return _PROGRAM_CACHE["nc"]


# ---------------------------------------------------------------------------
# Host-side entry point
# ---------------------------------------------------------------------------


def _bf16(a):
    return np.asarray(np.asarray(a, dtype=np.float32),
                      dtype=ml_dtypes.bfloat16)


def _make_mask(r):
    i = np.arange(128)[:, None]
    j = np.arange(512)[None, :]
    return np.where(j > 128 * r + i, np.float32(-NEG), np.float32(0.0))


def _make_in_maps(x, Wq, Wk, Wv, Wo):
    x = np.asarray(x, dtype=np.float32)
    wq = _bf16(Wq)
    wk = _bf16(Wk)
    wv = _bf16(Wv)
    wo = _bf16(Wo)
    ident = np.eye(128, dtype=np.float32)


    in_maps = []
    for core in range(8):
        b, r = divmod(core, 4)
        xb = x[b]
        x_kvT = np.ascontiguousarray(_bf16(xb[512 * r:512 * (r + 1)]).T)
        chunks = [r, r + 4, r + 8, r + 12]
        x_q = np.concatenate([xb[128 * c:128 * (c + 1)] for c in chunks],
                             axis=0)
        x_qT = np.ascontiguousarray(_bf16(x_q).T)
        # fused-shard gather rows: core c's shard occupies rows
        # [256c, 256c+256) of kv_gath; kt chunk at +0, v chunk at +128.
        p = np.arange(128, dtype=np.int32)
        gidx = np.empty((128, 8), dtype=np.int32)
        for g in range(4):
            gidx[:, g] = 256 * (4 * b + g) + p
            gidx[:, 4 + g] = 256 * (4 * b + g) + 128 + p
        in_maps.append({
            "x_kvt": x_kvT, "x_qt": x_qT,
            "wq": wq, "wk": wk, "wv": wv, "wo": wo,
            "mask": _make_mask(r), "ident": ident, "gidx": gidx,
        })
    return in_maps


def kernel(x, Wq, bq, Wk, bk, Wv, bv, Wo, bo, _bench=None):
    nc = _get_program()
    in_maps = _make_in_maps(x, Wq, Wk, Wv, Wo)

    kwargs = dict(_bench or {})
    res = run_bass_kernel_spmd(nc, in_maps, list(range(8)), **kwargs)

    out = np.empty((B, S, D), dtype=np.float32)
    for core in range(8):
        b, r = divmod(core, 4)
        yo = res.results[core]["y_out"]
        for i, c in enumerate([r, r + 4, r + 8, r + 12]):
            out[b, 128 * c:128 * (c + 1), :] = yo[128 * i:128 * (i + 1), :]
    if _bench is not None:
        kernel.last_result = res
    return out


kernel.last_result = None


# ---------------------------------------------------------------------------
# Benchmarking helper (used by test.py only): runs the kernel repeatedly
# through a persistent jitted PJRT executable with device-resident inputs,
# so per-call wall time approximates dispatch-overhead + HW exec time.
# ---------------------------------------------------------------------------


def make_runner(nc, in_maps, chain=1):
    """chain>1 invokes the NEFF that many times inside ONE jitted XLA
    program, feeding iteration i's outputs as iteration i+1's output
    seed buffers (true data dependence -> no CSE, serialized NEFF
    executions).  Per-call dispatch overhead is paid once, so
    (wall(chain=K) - wall(chain=1)) / (K-1) isolates HW exec time."""
    import jax
    from jax.sharding import Mesh, PartitionSpec, NamedSharding
    from jax.experimental.shard_map import shard_map
    from concourse.bass2jax import (
        _bass_exec_p, install_neuronx_cc_hook, partition_id_tensor,
    )

    install_neuronx_cc_hook()
    n_cores = len(in_maps)
    in_names, out_names, out_avals, zero_outs = [], [], [], []
    pname = nc.partition_id_tensor.name if nc.partition_id_tensor else None
    for alloc in nc.m.functions[0].allocations:
        if not isinstance(alloc, mybir.MemoryLocationSet):
            continue
        name = alloc.memorylocations[0].name
        if alloc.kind == "ExternalInput":
            if name != pname:
                in_names.append(name)
        elif alloc.kind == "ExternalOutput":
            shape = tuple(alloc.tensor_shape)
            dtype = mybir.dt.np(alloc.dtype)
            out_names.append(name)
            out_avals.append(jax.core.ShapedArray(shape, dtype))
            zero_outs.append(np.zeros(shape, dtype))
    n_params = len(in_names)
    all_in = list(in_names) + list(out_names)
    if pname:
        all_in.append(pname)

    def _body(*args):
        ins = list(args[:n_params])
        outs = tuple(args[n_params:])
        for _ in range(chain):
            operands = ins + list(outs)
            if pname is not None:
                operands.append(partition_id_tensor())
            outs = tuple(_bass_exec_p.bind(
                *operands, out_avals=tuple(out_avals), in_names=tuple(all_in),
                out_names=tuple(out_names), lowering_input_output_aliases=(),
                sim_require_finite=True, sim_require_nnan=True, nc=nc))
        return outs

    devices = jax.devices()[:n_cores]
    mesh = Mesh(np.asarray(devices), ("core",))
    specs_in = (PartitionSpec("core"),) * (n_params + len(out_names))
    specs_out = (PartitionSpec("core"),) * len(out_names)
    fn = jax.jit(shard_map(_body, mesh=mesh, in_specs=specs_in,
                           out_specs=specs_out, check_rep=False),
                 keep_unused=True)
    sh = NamedSharding(mesh, PartitionSpec("core"))
    concat_in = [np.concatenate([np.asarray(m[n]) for m in in_maps], axis=0)
                 for n in in_names]
    concat_zero = [np.zeros((n_cores * z.shape[0], *z.shape[1:]), z.dtype)
                   for z in zero_outs]
    dev_in = [jax.device_put(a, sh) for a in concat_in]
    dev_zero = [jax.device_put(a, sh) for a in concat_zero]
    return fn, dev_in, dev_zero, out_names


def bench_hw(inputs, iters=60, trials=3):
    """Pipelined-dispatch wall-clock per call for the real 8-core program
    and for a trivial null program; the difference estimates HW exec time."""
    import time
    import jax

    in_maps = _make_in_maps(inputs["x"], inputs["Wq"], inputs["Wk"],
                            inputs["Wv"], inputs["Wo"])

    def null_program():
        nnc = bass.Bass("TRN2", target_bir_lowering=False, debug=False,
                        num_devices=8)
        xi = nnc.dram_tensor("xn", [128, 128], F32, kind="ExternalInput")
        yo = nnc.dram_tensor("yn", [128, 128], F32, kind="ExternalOutput")
        with tile.TileContext(nnc) as tcc:
            with tcc.tile_pool(name="s", bufs=1) as pl:
                t = pl.tile([128, 128], F32, name="t0")
                nnc.sync.dma_start(t[:], xi.ap())
                nnc.sync.dma_start(yo.ap(), t[:])
        _split_waits(nnc)
        return nnc, [{"xn": np.zeros((128, 128), np.float32)}] * 8

    def measure(fn, di, dz):
        out = fn(*di, *dz)
        jax.block_until_ready(out)
        best = float("inf")
        for _ in range(trials):
            t0 = time.perf_counter()
            outs = [fn(*di, *dz) for _ in range(iters)]
            jax.block_until_ready(outs)
            dt = (time.perf_counter() - t0) / iters
            best = min(best, dt)
        return best

    nnc, null_maps = null_program()
    fn0, di0, dz0, _ = make_runner(nnc, null_maps)
    t_null = measure(fn0, di0, dz0)
    fn1, di1, dz1, _ = make_runner(_get_program(), in_maps)
    t_full = measure(fn1, di1, dz1)
    return t_full, t_null


# revision 38
# speedup vs baseline: 1.9107x; 1.0053x over previous
"""Trainium2 Bass kernel for nn_MultiHeadAttention_32066225832689.

Reference computation (B=2, S=2048, D=1024, fp32):
    q = relu(x @ Wq + bq); k = relu(x @ Wk + bk); v = relu(x @ Wv + bv)
    e = (q @ k^T) / sqrt(D);  e -= 10000 * causal_mask
    attn = softmax(e);  y = relu((attn @ v) @ Wo + bo)
Biases are all zeros by problem spec (fill: zeros) and are ignored.

Sharding over 8 NeuronCores: batch (2) x rank (4).  Rank r of a batch
group owns:
  - K/V projection for token rows [512r, 512r+512) (data-parallel),
    exchanged via TWO 8-core AllGathers with Shared-address-space
    outputs (the fast collective path; Shared output needs >4-core
    groups, and the 4-rank grouped AllGather takes the slow fold_n=2
    ring at ~77us/AG).  The K^T gather launches right after the K
    projection (~t=16us) so attention can start ~20us earlier; the V
    gather follows the V projection and completes behind the first
    attention blocks (SPLIT_CC=False selects a single fused-shard AG
    instead -- measured equal-median but ~13us slower best-case).
    The 8-core gather mixes both batch groups' chunks, so each core
    picks its batch's four chunks out of the gathered buffer with
    per-partition row-gather indirect DMAs whose row indices are
    host-provided per-core data -- the SPMD program stays identical
    on all cores.  (Static conditional DMAs gated on a host flag via
    values_load would avoid the SWDGE path, but the pinned walrus
    cannot encode the bounds-check register ISA: "ISA wrong length".)
  - Query chunks {r, r+4, r+8, r+12} (128 rows each).  Chunk c needs
    key chunks 0..c//4, so every rank processes blocks with 1,2,3,4
    key chunks of 512 -- a balanced, rank-uniform causal workload.

The whole data plane runs in bfloat16 (weights, x^T, K^T, V, Q^T, P):
same 1 cycle/row PE rate as fp32r but half the DMA/collective bytes
and SBUF footprint.  Scores/softmax/output accumulation stay fp32.
Numpy modeling of this rounding placement predicts rel err ~4e-3 and
hardware measures 2.0e-3 against the fp32 reference (budget 2e-2).
fp8e4m3 K/V was evaluated numerically at 3.7e-2 -- over budget.

x^T is pre-transposed on the host (free), removing v1's on-device PE
transpose stage.  The row max is accumulated incrementally per score
chunk (hidden behind the next chunk's matmuls) and 1/rowsum is folded
into the final relu's per-partition scale.  Weights/x stream on the
sync HWDGE ring (first x^T slice and first wk slice lead; consts
trail), bounce writes + output stores on the scalar ring, collective +
gathered-chunk loads on the gpsimd ring -- no input stream ever waits
on the collective.

Measured on TRN2 via reps=17-vs-9 back-to-back NEFF differencing at
150-iter pipelined dispatch (the only profiling on this axon build):
median 188 us/body over 6 rounds, best round 148 us (the terminal
showed +-15% background-load wobble late in the session; the fused-AG
variant measured median 189 / best 161 under the same conditions).
Collective-free variant 154 us; TimelineSim cost model 138 us.  v1
(fp32r, 4-rank AGs) measured 359 us +-1%.  Max rel err 2.017e-3.
"""

import sys

sys.path.insert(0, "/opt/trn_rl_repo")

import numpy as np
import ml_dtypes

import concourse.bass as bass
import concourse.mybir as mybir
from concourse import tile
from concourse.bass_utils import run_bass_kernel_spmd

F32 = mybir.dt.float32
BF16 = mybir.dt.bfloat16
I32 = mybir.dt.int32

B, S, D = 2, 2048, 1024
NEG = 10000.0
SCALE = 1.0 / 32.0  # 1/sqrt(D)

# Split the K^T/V exchange into two AllGathers (K first, launched right
# after the K projection) instead of one fused AG after V.  The K-AG
# lands ~20us earlier, so attention starts sooner; the V-AG completes
# behind the first attention blocks.  Host-side gidx depends on this.
SPLIT_CC = True

# ---------------------------------------------------------------------------
# Post-scheduling pass: split multi-wait instructions into NOP chains.
# The pinned walrus codegen accepts only one embedded sync-wait per
# instruction on most engine instruction formats; Tile's semaphore
# assignment freely emits several.  Rewrite each instruction with k>1
# waits into (k-1) same-engine NoOps carrying one wait each, inserted
# immediately before it (same engine program order => semantics kept).
# ---------------------------------------------------------------------------
_WSPLIT_CTR = [0]


def _split_waits(nc, max_waits=1):
    n = 0
    for f in nc.m.functions:
        for blk in f.blocks:
            out = []
            for inst in blk.instructions:
                si = inst.sync_info
                if si is not None and len(si.on_wait) > max_waits:
                    waits = list(si.on_wait)
                    for w in waits[:-max_waits]:
                        _WSPLIT_CTR[0] += 1
                        nop = mybir.InstNoOp(name=f"WSPLIT-{_WSPLIT_CTR[0]}")
                        nop.engine = inst.engine
                        nop.sync_info = mybir.SyncInfo(on_wait=[w], on_update=[])
                        out.append(nop)
                    inst.sync_info = mybir.SyncInfo(
                        on_wait=waits[-max_waits:], on_update=list(si.on_update)
                    )
                    n += 1
                out.append(inst)
            blk.instructions = out
    return n


# ---------------------------------------------------------------------------
# Kernel program (identical on all 8 cores)
# ---------------------------------------------------------------------------


def _build_program(timing=False, reps=1, no_cc=False):
    """timing=True builds a single-core variant (no collective; gathered
    K^T/V reads redirected to the local bounce buffer) with identical
    instruction mix/volume, for TimelineSim cost-model analysis.
    reps>1 emits the whole kernel body that many times back-to-back
    (benchmarking: amortizes the per-dispatch overhead)."""
    nc = bass.Bass(
        "TRN2", target_bir_lowering=False, debug=False,
        num_devices=1 if timing else 8,
    )

    x_kvT = nc.dram_tensor("x_kvt", [D, 512], BF16, kind="ExternalInput")
    x_qT = nc.dram_tensor("x_qt", [D, 512], BF16, kind="ExternalInput")
    wq_in = nc.dram_tensor("wq", [D, D], BF16, kind="ExternalInput")
    wk_in = nc.dram_tensor("wk", [D, D], BF16, kind="ExternalInput")
    wv_in = nc.dram_tensor("wv", [D, D], BF16, kind="ExternalInput")
    wo_in = nc.dram_tensor("wo", [D, D], BF16, kind="ExternalInput")
    mask_in = nc.dram_tensor("mask", [128, 512], F32, kind="ExternalInput")
    ident_in = nc.dram_tensor("ident", [128, 128], F32, kind="ExternalInput")
    gidx_in = nc.dram_tensor("gidx", [128, 8], I32, kind="ExternalInput")
    y_out = nc.dram_tensor("y_out", [512, D], F32, kind="ExternalOutput")

    with tile.TileContext(nc) as tc:
        for _rep in range(reps):
            _emit(nc, tc, x_kvT, x_qT, wq_in, wk_in, wv_in, wo_in, mask_in,
                  ident_in, gidx_in, y_out, timing or no_cc)

    _split_waits(nc)
    return nc


SECTIONS = []


def _emit(nc, tc, x_kvT, x_qT, wq_in, wk_in, wv_in, wo_in, mask_in, ident_in,
          gidx_in, y_out, timing):
    del SECTIONS[:]

    def mark(label):
        try:
            SECTIONS.append((nc.next_id(), label))
        except Exception:
            pass

    Relu = mybir.ActivationFunctionType.Relu
    Exp = mybir.ActivationFunctionType.Exp
    AX = mybir.AxisListType.X

    pools = []

    def pool(name, bufs, space="SBUF"):
        p = tc.alloc_tile_pool(name=name, bufs=bufs, space=space)
        pools.append(p)
        return p

    # ----- long-lived pools -----
    const_p = pool("const", 1)
    qt_p = pool("qt", 1)
    wo_p = pool("wo", 1)
    e_p = pool("e", 1)
    pt_p = pool("pt", 2)
    y_p = pool("y", 1)
    yt_p = pool("yt", 1)
    out_p = pool("out", 1)
    st_p = pool("st", 2)
    res_p = pool("res", 1)
    dram_p = pool("dram", 1, space="DRAM")

    # const tiles (DMAs emitted later, after the weight streams, so they
    # never delay the first projection matmuls on the sync ring)
    ident_t = const_p.tile([128, 128], F32, tag="ident")
    mask_t = const_p.tile([128, 512], F32, tag="mask")
    gidx_t = const_p.tile([128, 8], I32, tag="gidx")

    qt_t = qt_p.tile([128, 4096], BF16, tag="qt")   # [dout d-tile, 512 q-tok]
    wo_t = wo_p.tile([128, 8192], BF16, tag="wo")   # [din d-tile, 1024 dout]

    # gathered K^T / V residency: all 4 chunks (incl. own) come back from
    # the gathered buffer via indirect DMA -- the own-chunk's gather row
    # position is per-core data, so the program stays core-agnostic.
    kt_res = [res_p.tile([128, 4096], BF16, tag=f"kt{g}", name=f"kt_res{g}")
              for g in range(4)]
    v_res = [res_p.tile([128, 4096], BF16, tag=f"v{g}", name=f"v_res{g}")
             for g in range(4)]

    # collective bounce buffers (DRAM pool tiles -> Tile tracks deps).
    # Shard = [128 rows, 4096 bf16] per tensor (8 KiB rows), so a
    # gathered chunk reload is ONE per-partition row-gather DMA.
    if SPLIT_CC:
        kt_in = dram_p.tile([128, 4096], BF16, tag="kt_in")
        v_in = dram_p.tile([128, 4096], BF16, tag="v_in")
        if not timing:
            kt_gath = dram_p.tile([1024, 4096], BF16, tag="kt_gath",
                                  addr_space="Shared")
            v_gath = dram_p.tile([1024, 4096], BF16, tag="v_gath",
                                 addr_space="Shared")
        else:
            kt_gath, v_gath = kt_in, v_in
    else:
        kv_in = dram_p.tile([256, 4096], BF16, tag="kv_in")
        if not timing:
            kv_gath = dram_p.tile([2048, 4096], BF16, tag="kv_gath",
                                  addr_space="Shared")
        else:
            kv_gath = kv_in
        kt_in = kv_in[0:128, :]
        v_in = kv_in[128:256, :]
        kt_gath = v_gath = kv_gath

    def load_chunk(dst, src, col):
        nc.gpsimd.indirect_dma_start(
            out=dst[:, :],
            out_offset=None,
            in_=src[:, :],
            in_offset=bass.IndirectOffsetOnAxis(ap=gidx_t[:, col:col + 1],
                                                axis=0),
        )

    # =====================================================================
    # Projections: K^T own -> V own -> fused AllGather -> Q^T + Wo
    # =====================================================================
    with tc.tile_pool(name="pP", bufs=1) as pp, \
         tc.tile_pool(name="wstream", bufs=8) as wsp, \
         tc.tile_pool(name="ps_pmm", bufs=8, space="PSUM") as ps_mm:

        xt_kv = pp.tile([128, 4096], BF16, tag="xt_kv")
        xt_q = pp.tile([128, 4096], BF16, tag="xt_q")
        kt_own = pp.tile([128, 4096], BF16, tag="kt_own")
        v_own = pp.tile([128, 4096], BF16, tag="v_own")

        # first-MM-critical DMAs lead the sync ring: x^T d-slice 0, then
        # the first wk slice arrives inside the K-proj loop below.
        mark("xt_kv")
        xkv3 = xt_kv.rearrange("p (d b) -> p d b", b=512)
        nc.sync.dma_start(xkv3[:, 0, :], x_kvT.ap()[0:128, :])

        # ---- K^T own: kt_own[p, 512m+tok] = relu(K^T)[128m+p, tok]
        mark("kT")
        for half in range(2):
            mms = [ps_mm.tile([128, 512], F32, tag="mm", name=f"mmk{half}_{i}")
                   for i in range(4)]
            for d in range(8):
                wk_d = wsp.tile([128, 512], BF16, tag="w", name=f"wk{half}{d}")
                nc.sync.dma_start(
                    wk_d[:], wk_in.ap()[128 * d:128 * (d + 1),
                                        512 * half:512 * (half + 1)])
                if half == 0 and d < 7:
                    nc.sync.dma_start(xkv3[:, d + 1, :],
                                      x_kvT.ap()[128 * (d + 1):128 * (d + 2), :])
                for mi in range(4):
                    nc.tensor.matmul(
                        mms[mi][:],
                        wk_d[:, 128 * mi:128 * (mi + 1)],
                        xt_kv[:, 512 * d:512 * (d + 1)],
                        start=(d == 0), stop=(d == 7),
                    )
            for mi in range(4):
                m = 4 * half + mi
                nc.scalar.activation(kt_own[:, 512 * m:512 * (m + 1)],
                                     mms[mi][:], Relu)
        nc.scalar.dma_start(kt_in[:, :], kt_own[:, :])

        mark("cc_kt")
        if SPLIT_CC and not timing:
            nc.gpsimd.collective_compute(
                "AllGather",
                mybir.AluOpType.bypass,
                replica_groups=[[0, 1, 2, 3, 4, 5, 6, 7]],
                ins=[kt_in[:, :]],
                outs=[kt_gath[:, :]],
            )

        # ---- V own: v_own[p, 1024t+dv] = relu(V)[128t+p, dv]
        mark("V")
        for h in range(2):
            mms = [ps_mm.tile([128, 512], F32, tag="mm", name=f"mmv{h}_{i}")
                   for i in range(4)]
            for d in range(8):
                wv_d = wsp.tile([128, 512], BF16, tag="w", name=f"wv{h}{d}")
                nc.sync.dma_start(
                    wv_d[:], wv_in.ap()[128 * d:128 * (d + 1),
                                        512 * h:512 * (h + 1)])
                for t in range(4):
                    nc.tensor.matmul(
                        mms[t][:],
                        xt_kv[:, 512 * d + 128 * t:512 * d + 128 * (t + 1)],
                        wv_d[:],
                        start=(d == 0), stop=(d == 7),
                    )
            for t in range(4):
                nc.scalar.activation(
                    v_own[:, 1024 * t + 512 * h:1024 * t + 512 * (h + 1)],
                    mms[t][:], Relu)
        nc.scalar.dma_start(v_in[:, :], v_own[:, :])

        mark("consts")
        nc.sync.dma_start(gidx_t[:], gidx_in.ap())
        nc.sync.dma_start(ident_t[:], ident_in.ap())
        nc.sync.dma_start(mask_t[:], mask_in.ap())

        mark("cc_v")
        if not timing:
            if SPLIT_CC:
                nc.gpsimd.collective_compute(
                    "AllGather",
                    mybir.AluOpType.bypass,
                    replica_groups=[[0, 1, 2, 3, 4, 5, 6, 7]],
                    ins=[v_in[:, :]],
                    outs=[v_gath[:, :]],
                )
            else:
                nc.gpsimd.collective_compute(
                    "AllGather",
                    mybir.AluOpType.bypass,
                    replica_groups=[[0, 1, 2, 3, 4, 5, 6, 7]],
                    ins=[kv_in[:, :]],
                    outs=[kv_gath[:, :]],
                )

        mark("xt_q")
        xq3 = xt_q.rearrange("p (d b) -> p d b", b=512)
        for d in range(8):
            nc.sync.dma_start(xq3[:, d, :],
                              x_qT.ap()[128 * d:128 * (d + 1), :])

        # ---- Q^T own (scaled by 1/sqrt(D))
        mark("qT")
        for half in range(2):
            mms = [ps_mm.tile([128, 512], F32, tag="mm", name=f"mmq{half}_{i}")
                   for i in range(4)]
            for d in range(8):
                wq_d = wsp.tile([128, 512], BF16, tag="w", name=f"wq{half}{d}")
                nc.sync.dma_start(
                    wq_d[:], wq_in.ap()[128 * d:128 * (d + 1),
                                        512 * half:512 * (half + 1)])
                for mi in range(4):
                    nc.tensor.matmul(
                        mms[mi][:],
                        wq_d[:, 128 * mi:128 * (mi + 1)],
                        xt_q[:, 512 * d:512 * (d + 1)],
                        start=(d == 0), stop=(d == 7),
                    )
            for mi in range(4):
                m = 4 * half + mi
                nc.scalar.activation(qt_t[:, 512 * m:512 * (m + 1)],
                                     mms[mi][:], Relu, scale=SCALE)

        mark("woload")
        for d in range(8):
            nc.sync.dma_start(wo_t[:, 1024 * d:1024 * (d + 1)],
                              wo_in.ap()[128 * d:128 * (d + 1), :])

        # gathered chunk loads (gpsimd ring, behind the AG trigger(s)),
        # in consumption order.  Split mode: all kt loads first (they
        # only wait on the early K-AG; a v load would block the queue on
        # the V-AG), then v loads.
        mark("kvload")
        if SPLIT_CC:
            order = [("kt", 0), ("kt", 1), ("kt", 2), ("kt", 3),
                     ("v", 0), ("v", 1), ("v", 2), ("v", 3)]
        else:
            order = [("kt", 0), ("v", 0), ("kt", 1), ("kt", 2),
                     ("v", 1), ("kt", 3), ("v", 2), ("v", 3)]
        for kind, g in order:
            if kind == "kt":
                load_chunk(kt_res[g], kt_gath, g)
            else:
                load_chunk(v_res[g], v_gath, 4 + g)

    # =====================================================================
    # Attention + output projection, software-pipelined across blocks
    # =====================================================================
    with tc.tile_pool(name="ps_cmm", bufs=2, space="PSUM") as ps_mm, \
         tc.tile_pool(name="ps_ctr", bufs=2, space="PSUM") as ps_tr, \
         tc.tile_pool(name="ps_y", bufs=1, space="PSUM") as ps_y, \
         tc.tile_pool(name="ps_yt", bufs=1, space="PSUM") as ps_yt:

        st = {}

        def emit_e(i):
            mark(f"e{i}")
            if i == 0:
                # the [128,1024] output-staging slot is idle until tail0 and
                # block0's scores die before tail0 -- reuse it so block1's
                # copies never wait on block0's transposes.
                e_t = out_p.tile([128, 1024], F32, tag="osb", name="e0")[:, 0:512]
            else:
                e_t = e_p.tile([128, 512 * (i + 1)], F32,
                               tag=("eA" if i < 2 else "eB"), name=f"e{i}")
            st[i] = {"e": e_t}
            negmax = None
            for g in range(i + 1):
                mm = ps_mm.tile([128, 512], F32, tag="mm", name=f"mme{i}{g}")
                ktg = kt_res[g].rearrange("p (d b) -> p d b", b=512)
                for d in range(8):
                    nc.tensor.matmul(
                        mm[:],
                        qt_t[:, 512 * d + 128 * i:512 * d + 128 * (i + 1)],
                        ktg[:, d, :],
                        start=(d == 0), stop=(d == 7),
                    )
                if g == i:
                    nc.vector.tensor_add(e_t[:, 512 * g:512 * (g + 1)],
                                         mm[:], mask_t[:])
                else:
                    nc.vector.tensor_copy(e_t[:, 512 * g:512 * (g + 1)], mm[:])
                # incremental per-chunk negated max, hidden behind the next
                # chunk's score matmuls: negmax = min_g(-max(chunk_g))
                nm_g = st_p.tile([128, 1], F32, tag="nmg", name=f"nm{i}_{g}")
                nc.vector.reduce_max(nm_g[:], e_t[:, 512 * g:512 * (g + 1)],
                                     axis=AX, negate=True)
                if negmax is None:
                    negmax = nm_g
                else:
                    acc = st_p.tile([128, 1], F32, tag="nmacc",
                                    name=f"nma{i}_{g}")
                    nc.vector.tensor_tensor(acc[:], negmax[:], nm_g[:],
                                            op=mybir.AluOpType.min)
                    negmax = acc
            st[i]["negmax"] = negmax

        def emit_softmax(i):
            mark(f"sm{i}")
            e_t = st[i]["e"]
            W = 512 * (i + 1)
            rowsum = st_p.tile([128, 1], F32, tag="rowsum", name=f"rs{i}")
            nc.scalar.activation(e_t[:, 0:W], e_t[:, 0:W], Exp,
                                 bias=st[i]["negmax"][:], scale=1.0,
                                 accum_out=rowsum[:])
            rinv = st_p.tile([128, 1], F32, tag="rinv", name=f"ri{i}")
            nc.vector.reciprocal(rinv[:], rowsum[:])
            st[i]["rinv"] = rinv

        def emit_trav(i):
            mark(f"av{i}")
            e_t = st[i]["e"]
            yps = ps_y.tile([128, 1024], F32, tag="yacc", name=f"y{i}")
            st[i]["yps"] = yps
            for g in range(i + 1):
                trp = ps_tr.tile([128, 512], F32, tag="ctr", name=f"ctr{i}{g}")
                for j in range(4):
                    nc.tensor.transpose(
                        trp[:, 128 * j:128 * (j + 1)],
                        e_t[:, 512 * g + 128 * j:512 * g + 128 * (j + 1)],
                        ident_t[:],
                    )
                pt_t = pt_p.tile([128, 512], BF16, tag="pt", name=f"pt{i}{g}")
                nc.vector.tensor_copy(pt_t[:], trp[:])
                vg = v_res[g].rearrange("p (t b) -> p t b", b=1024)
                for j in range(4):
                    for h in range(2):
                        nc.tensor.matmul(
                            yps[:, 512 * h:512 * (h + 1)],
                            pt_t[:, 128 * j:128 * (j + 1)],
                            vg[:, j, 512 * h:512 * (h + 1)],
                            start=(g == 0 and j == 0),
                            stop=(g == i and j == 3),
                        )

        def emit_tail(i):
            # y stays unnormalized; 1/rowsum is applied as the per-partition
            # scale of the final relu (relu(a*c) = relu(a)*c for c > 0).
            mark(f"tail{i}")
            y_t = y_p.tile([128, 1024], F32, tag="ysb", name=f"ysb{i}")
            nc.vector.tensor_copy(y_t[:, 0:512], st[i]["yps"][:, 0:512])
            nc.vector.tensor_copy(y_t[:, 512:1024], st[i]["yps"][:, 512:1024])
            ytp = ps_yt.tile([128, 1024], F32, tag="ytp", name=f"ytp{i}")
            for d in range(8):
                nc.tensor.transpose(
                    ytp[:, 128 * d:128 * (d + 1)],
                    y_t[:, 128 * d:128 * (d + 1)],
                    ident_t[:],
                )
            yt_t = yt_p.tile([128, 1024], BF16, tag="ytsb", name=f"ytsb{i}")
            nc.vector.tensor_copy(yt_t[:], ytp[:])
            o_t = out_p.tile([128, 1024], F32, tag="osb", name=f"osb{i}")
            for h in range(2):
                mm = ps_mm.tile([128, 512], F32, tag="mm", name=f"mmo{i}{h}")
                for d in range(8):
                    nc.tensor.matmul(
                        mm[:],
                        yt_t[:, 128 * d:128 * (d + 1)],
                        wo_t[:, 1024 * d + 512 * h:1024 * d + 512 * (h + 1)],
                        start=(d == 0), stop=(d == 7),
                    )
                nc.scalar.activation(o_t[:, 512 * h:512 * (h + 1)], mm[:], Relu,
                                     scale=st[i]["rinv"][:])
                # per-half store: the second half's relu overlaps the first
                # half's writeback, shortening the final-block drain.
                nc.scalar.dma_start(
                    y_out.ap()[128 * i:128 * (i + 1), 512 * h:512 * (h + 1)],
                    o_t[:, 512 * h:512 * (h + 1)])

        # pipelined emission: PE fills softmax bubbles with the next
        # block's score matmuls.
        emit_e(0)
        emit_softmax(0)
        emit_e(1)
        emit_trav(0)
        emit_tail(0)
        emit_softmax(1)
        emit_e(2)
        emit_trav(1)
        emit_tail(1)
        emit_softmax(2)
        emit_e(3)
        emit_trav(2)
        emit_tail(2)
        emit_softmax(3)
        emit_trav(3)
        emit_tail(3)

    mark("end")
    for p in reversed(pools):
        p.release()


_PROGRAM_CACHE = {}


def _get_program():
    if "nc" not in _PROGRAM_CACHE:
        _PROGRAM_CACHE["nc"] = _build_program()
    return _PROGRAM_CACHE["nc"]


# ---------------------------------------------------------------------------
# Host-side entry point
# ---------------------------------------------------------------------------


def _bf16(a):
    return np.asarray(np.asarray(a, dtype=np.float32),
                      dtype=ml_dtypes.bfloat16)


def _make_mask(r):
    i = np.arange(128)[:, None]
    j = np.arange(512)[None, :]
    return np.where(j > 128 * r + i, np.float32(-NEG), np.float32(0.0))


def _make_in_maps(x, Wq, Wk, Wv, Wo):
    x = np.asarray(x, dtype=np.float32)
    wq = _bf16(Wq)
    wk = _bf16(Wk)
    wv = _bf16(Wv)
    wo = _bf16(Wo)
    ident = np.eye(128, dtype=np.float32)


    in_maps = []
    for core in range(8):
        b, r = divmod(core, 4)
        xb = x[b]
        x_kvT = np.ascontiguousarray(_bf16(xb[512 * r:512 * (r + 1)]).T)
        chunks = [r, r + 4, r + 8, r + 12]
        x_q = np.concatenate([xb[128 * c:128 * (c + 1)] for c in chunks],
                             axis=0)
        x_qT = np.ascontiguousarray(_bf16(x_q).T)
        # gather rows for the chunk reloads.  Split mode: chunk g of this
        # core's batch sits at rows [128*(4b+g), ...) of each gather
        # buffer.  Fused mode: core shards are [256c, 256c+256) of
        # kv_gath with kt at +0 and v at +128.
        p = np.arange(128, dtype=np.int32)
        gidx = np.empty((128, 8), dtype=np.int32)
        for g in range(4):
            if SPLIT_CC:
                gidx[:, g] = 128 * (4 * b + g) + p
                gidx[:, 4 + g] = 128 * (4 * b + g) + p
            else:
                gidx[:, g] = 256 * (4 * b + g) + p
                gidx[:, 4 + g] = 256 * (4 * b + g) + 128 + p
        in_maps.append({
            "x_kvt": x_kvT, "x_qt": x_qT,
            "wq": wq, "wk": wk, "wv": wv, "wo": wo,
            "mask": _make_mask(r), "ident": ident, "gidx": gidx,
        })
    return in_maps


def kernel(x, Wq, bq, Wk, bk, Wv, bv, Wo, bo, _bench=None):
    nc = _get_program()
    in_maps = _make_in_maps(x, Wq, Wk, Wv, Wo)

    kwargs = dict(_bench or {})
    res = run_bass_kernel_spmd(nc, in_maps, list(range(8)), **kwargs)

    out = np.empty((B, S, D), dtype=np.float32)
    for core in range(8):
        b, r = divmod(core, 4)
        yo = res.results[core]["y_out"]
        for i, c in enumerate([r, r + 4, r + 8, r + 12]):
            out[b, 128 * c:128 * (c + 1), :] = yo[128 * i:128 * (i + 1), :]
    if _bench is not None:
        kernel.last_result = res
    return out


kernel.last_result = None


# ---------------------------------------------------------------------------
# Benchmarking helper (used by test.py only): runs the kernel repeatedly
# through a persistent jitted PJRT executable with device-resident inputs,
# so per-call wall time approximates dispatch-overhead + HW exec time.
# ---------------------------------------------------------------------------


def make_runner(nc, in_maps, chain=1):
    """chain>1 invokes the NEFF that many times inside ONE jitted XLA
    program, feeding iteration i's outputs as iteration i+1's output
    seed buffers (true data dependence -> no CSE, serialized NEFF
    executions).  Per-call dispatch overhead is paid once, so
    (wall(chain=K) - wall(chain=1)) / (K-1) isolates HW exec time."""
    import jax
    from jax.sharding import Mesh, PartitionSpec, NamedSharding
    from jax.experimental.shard_map import shard_map
    from concourse.bass2jax import (
        _bass_exec_p, install_neuronx_cc_hook, partition_id_tensor,
    )

    install_neuronx_cc_hook()
    n_cores = len(in_maps)
    in_names, out_names, out_avals, zero_outs = [], [], [], []
    pname = nc.partition_id_tensor.name if nc.partition_id_tensor else None
    for alloc in nc.m.functions[0].allocations:
        if not isinstance(alloc, mybir.MemoryLocationSet):
            continue
        name = alloc.memorylocations[0].name
        if alloc.kind == "ExternalInput":
            if name != pname:
                in_names.append(name)
        elif alloc.kind == "ExternalOutput":
            shape = tuple(alloc.tensor_shape)
            dtype = mybir.dt.np(alloc.dtype)
            out_names.append(name)
            out_avals.append(jax.core.ShapedArray(shape, dtype))
            zero_outs.append(np.zeros(shape, dtype))
    n_params = len(in_names)
    all_in = list(in_names) + list(out_names)
    if pname:
        all_in.append(pname)

    def _body(*args):
        ins = list(args[:n_params])
        outs = tuple(args[n_params:])
        for _ in range(chain):
            operands = ins + list(outs)
            if pname is not None:
                operands.append(partition_id_tensor())
            outs = tuple(_bass_exec_p.bind(
                *operands, out_avals=tuple(out_avals), in_names=tuple(all_in),
                out_names=tuple(out_names), lowering_input_output_aliases=(),
                sim_require_finite=True, sim_require_nnan=True, nc=nc))
        return outs

    devices = jax.devices()[:n_cores]
    mesh = Mesh(np.asarray(devices), ("core",))
    specs_in = (PartitionSpec("core"),) * (n_params + len(out_names))
    specs_out = (PartitionSpec("core"),) * len(out_names)
    fn = jax.jit(shard_map(_body, mesh=mesh, in_specs=specs_in,
                           out_specs=specs_out, check_rep=False),
                 keep_unused=True)
    sh = NamedSharding(mesh, PartitionSpec("core"))
    concat_in = [np.concatenate([np.asarray(m[n]) for m in in_maps], axis=0)
                 for n in in_names]
    concat_zero = [np.zeros((n_cores * z.shape[0], *z.shape[1:]), z.dtype)
                   for z in zero_outs]
    dev_in = [jax.device_put(a, sh) for a in concat_in]
    dev_zero = [jax.device_put(a, sh) for a in concat_zero]
    return fn, dev_in, dev_zero, out_names


def bench_hw(inputs, iters=60, trials=3):
    """Pipelined-dispatch wall-clock per call for the real 8-core program
    and for a trivial null program; the difference estimates HW exec time."""
    import time
    import jax

    in_maps = _make_in_maps(inputs["x"], inputs["Wq"], inputs["Wk"],
                            inputs["Wv"], inputs["Wo"])

    def null_program():
        nnc = bass.Bass("TRN2", target_bir_lowering=False, debug=False,
                        num_devices=8)
        xi = nnc.dram_tensor("xn", [128, 128], F32, kind="ExternalInput")
        yo = nnc.dram_tensor("yn", [128, 128], F32, kind="ExternalOutput")
        with tile.TileContext(nnc) as tcc:
            with tcc.tile_pool(name="s", bufs=1) as pl:
                t = pl.tile([128, 128], F32, name="t0")
                nnc.sync.dma_start(t[:], xi.ap())
                nnc.sync.dma_start(yo.ap(), t[:])
        _split_waits(nnc)
        return nnc, [{"xn": np.zeros((128, 128), np.float32)}] * 8

    def measure(fn, di, dz):
        out = fn(*di, *dz)
        jax.block_until_ready(out)
        best = float("inf")
        for _ in range(trials):
            t0 = time.perf_counter()
            outs = [fn(*di, *dz) for _ in range(iters)]
            jax.block_until_ready(outs)
            dt = (time.perf_counter() - t0) / iters
            best = min(best, dt)
        return best

    nnc, null_maps = null_program()
    fn0, di0, dz0, _ = make_runner(nnc, null_maps)
    t_null = measure(fn0, di0, dz0)
    fn1, di1, dz1, _ = make_runner(_get_program(), in_maps)
    t_full = measure(fn1, di1, dz1)
    return t_full, t_null


# revision 43
# speedup vs baseline: 2.0644x; 1.0805x over previous
"""Trainium2 Bass kernel for nn_MultiHeadAttention_32066225832689.

Reference computation (B=2, S=2048, D=1024, fp32):
    q = relu(x @ Wq + bq); k = relu(x @ Wk + bk); v = relu(x @ Wv + bv)
    e = (q @ k^T) / sqrt(D);  e -= 10000 * causal_mask
    attn = softmax(e);  y = relu((attn @ v) @ Wo + bo)
Biases are all zeros by problem spec (fill: zeros) and are ignored.

Sharding over 8 NeuronCores: batch (2) x rank (4).  Rank r of a batch
group owns:
  - K/V projection for token rows [512r, 512r+512) (data-parallel),
    exchanged via TWO 8-core AllGathers with Shared-address-space
    outputs (the fast collective path; Shared output needs >4-core
    groups, and the 4-rank grouped AllGather takes the slow fold_n=2
    ring at ~77us/AG).  The K^T gather launches right after the K
    projection (~t=16us) so attention can start ~20us earlier; the V
    gather follows the V projection and completes behind the first
    attention blocks (SPLIT_CC=False selects a single fused-shard AG
    instead -- measured equal-median but ~13us slower best-case).
    The 8-core gather mixes both batch groups' chunks, so each core
    picks its batch's four chunks out of the gathered buffer with
    per-partition row-gather indirect DMAs whose row indices are
    host-provided per-core data -- the SPMD program stays identical
    on all cores.  (Static conditional DMAs gated on a host flag via
    values_load would avoid the SWDGE path, but the pinned walrus
    cannot encode the bounds-check register ISA: "ISA wrong length".)
  - Query chunks {r, r+4, r+8, r+12} (128 rows each).  Chunk c needs
    key chunks 0..c//4, so every rank processes blocks with 1,2,3,4
    key chunks of 512 -- a balanced, rank-uniform causal workload.

The whole data plane runs in bfloat16 (weights, x^T, K^T, V, Q^T, P):
same 1 cycle/row PE rate as fp32r but half the DMA/collective bytes
and SBUF footprint.  Scores/softmax/output accumulation stay fp32.
Numpy modeling of this rounding placement predicts rel err ~4e-3 and
hardware measures 2.0e-3 against the fp32 reference (budget 2e-2).
fp8e4m3 K/V was evaluated numerically at 3.7e-2 -- over budget.

x^T is pre-transposed on the host (free), removing v1's on-device PE
transpose stage.  The row max is accumulated incrementally per score
chunk (hidden behind the next chunk's matmuls) and 1/rowsum is folded
into the final relu's per-partition scale.  Weights/x stream on the
sync HWDGE ring (first x^T slice and first wk slice lead; consts
trail), bounce writes + output stores on the scalar ring, collective +
gathered-chunk loads on the gpsimd ring -- no input stream ever waits
on the collective.  Attention blocks run in order [0, 2, 3, 1]:
block 0 first (needs only kt chunk 0 -> earliest start after the
K-AG), block 1 last so the exposed serial tail of the final block
(scores -> softmax -> AV -> out-proj) is near-minimal instead of
maximal; y^T evacuation is half-granular so out-proj matmuls overlap
the remaining transposes.

Measured on TRN2 via reps=17-vs-9 back-to-back NEFF differencing at
150-iter pipelined dispatch (the only profiling on this axon build):
median 174 us/body, best round 137 us (the terminal showed +-15%
background-load wobble late in the session; in-order blocks with the
same split AGs measured median 188 / best 148, the fused-AG variant
189/161).  Collective-free variant 154 us; TimelineSim cost model
138 us.  v1 (fp32r, 4-rank AGs) measured 359 us +-1%.  Max rel err
2.017e-3.
"""

import sys

sys.path.insert(0, "/opt/trn_rl_repo")

import numpy as np
import ml_dtypes

import concourse.bass as bass
import concourse.mybir as mybir
from concourse import tile
from concourse.bass_utils import run_bass_kernel_spmd

F32 = mybir.dt.float32
BF16 = mybir.dt.bfloat16
I32 = mybir.dt.int32

B, S, D = 2, 2048, 1024
NEG = 10000.0
SCALE = 1.0 / 32.0  # 1/sqrt(D)

# Split the K^T/V exchange into two AllGathers (K first, launched right
# after the K projection) instead of one fused AG after V.  The K-AG
# lands ~20us earlier, so attention starts sooner; the V-AG completes
# behind the first attention blocks.  Host-side gidx depends on this.
SPLIT_CC = True

# ---------------------------------------------------------------------------
# Post-scheduling pass: split multi-wait instructions into NOP chains.
# The pinned walrus codegen accepts only one embedded sync-wait per
# instruction on most engine instruction formats; Tile's semaphore
# assignment freely emits several.  Rewrite each instruction with k>1
# waits into (k-1) same-engine NoOps carrying one wait each, inserted
# immediately before it (same engine program order => semantics kept).
# ---------------------------------------------------------------------------
_WSPLIT_CTR = [0]


def _split_waits(nc, max_waits=1):
    n = 0
    for f in nc.m.functions:
        for blk in f.blocks:
            out = []
            for inst in blk.instructions:
                si = inst.sync_info
                if si is not None and len(si.on_wait) > max_waits:
                    waits = list(si.on_wait)
                    for w in waits[:-max_waits]:
                        _WSPLIT_CTR[0] += 1
                        nop = mybir.InstNoOp(name=f"WSPLIT-{_WSPLIT_CTR[0]}")
                        nop.engine = inst.engine
                        nop.sync_info = mybir.SyncInfo(on_wait=[w], on_update=[])
                        out.append(nop)
                    inst.sync_info = mybir.SyncInfo(
                        on_wait=waits[-max_waits:], on_update=list(si.on_update)
                    )
                    n += 1
                out.append(inst)
            blk.instructions = out
    return n


# ---------------------------------------------------------------------------
# Kernel program (identical on all 8 cores)
# ---------------------------------------------------------------------------


def _build_program(timing=False, reps=1, no_cc=False):
    """timing=True builds a single-core variant (no collective; gathered
    K^T/V reads redirected to the local bounce buffer) with identical
    instruction mix/volume, for TimelineSim cost-model analysis.
    reps>1 emits the whole kernel body that many times back-to-back
    (benchmarking: amortizes the per-dispatch overhead)."""
    nc = bass.Bass(
        "TRN2", target_bir_lowering=False, debug=False,
        num_devices=1 if timing else 8,
    )

    x_kvT = nc.dram_tensor("x_kvt", [D, 512], BF16, kind="ExternalInput")
    x_qT = nc.dram_tensor("x_qt", [D, 512], BF16, kind="ExternalInput")
    wq_in = nc.dram_tensor("wq", [D, D], BF16, kind="ExternalInput")
    wk_in = nc.dram_tensor("wk", [D, D], BF16, kind="ExternalInput")
    wv_in = nc.dram_tensor("wv", [D, D], BF16, kind="ExternalInput")
    wo_in = nc.dram_tensor("wo", [D, D], BF16, kind="ExternalInput")
    mask_in = nc.dram_tensor("mask", [128, 512], F32, kind="ExternalInput")
    ident_in = nc.dram_tensor("ident", [128, 128], F32, kind="ExternalInput")
    gidx_in = nc.dram_tensor("gidx", [128, 8], I32, kind="ExternalInput")
    y_out = nc.dram_tensor("y_out", [512, D], F32, kind="ExternalOutput")

    with tile.TileContext(nc) as tc:
        for _rep in range(reps):
            _emit(nc, tc, x_kvT, x_qT, wq_in, wk_in, wv_in, wo_in, mask_in,
                  ident_in, gidx_in, y_out, timing or no_cc)

    _split_waits(nc)
    return nc


SECTIONS = []


def _emit(nc, tc, x_kvT, x_qT, wq_in, wk_in, wv_in, wo_in, mask_in, ident_in,
          gidx_in, y_out, timing):
    del SECTIONS[:]

    def mark(label):
        try:
            SECTIONS.append((nc.next_id(), label))
        except Exception:
            pass

    Relu = mybir.ActivationFunctionType.Relu
    Exp = mybir.ActivationFunctionType.Exp
    AX = mybir.AxisListType.X

    pools = []

    def pool(name, bufs, space="SBUF"):
        p = tc.alloc_tile_pool(name=name, bufs=bufs, space=space)
        pools.append(p)
        return p

    # ----- long-lived pools -----
    const_p = pool("const", 1)
    qt_p = pool("qt", 1)
    wo_p = pool("wo", 1)
    e_p = pool("e", 1)
    pt_p = pool("pt", 2)
    y_p = pool("y", 1)
    yt_p = pool("yt", 1)
    out_p = pool("out", 1)
    st_p = pool("st", 2)
    res_p = pool("res", 1)
    dram_p = pool("dram", 1, space="DRAM")

    # const tiles (DMAs emitted later, after the weight streams, so they
    # never delay the first projection matmuls on the sync ring)
    ident_t = const_p.tile([128, 128], F32, tag="ident")
    mask_t = const_p.tile([128, 512], F32, tag="mask")
    gidx_t = const_p.tile([128, 8], I32, tag="gidx")

    qt_t = qt_p.tile([128, 4096], BF16, tag="qt")   # [dout d-tile, 512 q-tok]
    wo_t = wo_p.tile([128, 8192], BF16, tag="wo")   # [din d-tile, 1024 dout]

    # gathered K^T / V residency: all 4 chunks (incl. own) come back from
    # the gathered buffer via indirect DMA -- the own-chunk's gather row
    # position is per-core data, so the program stays core-agnostic.
    kt_res = [res_p.tile([128, 4096], BF16, tag=f"kt{g}", name=f"kt_res{g}")
              for g in range(4)]
    v_res = [res_p.tile([128, 4096], BF16, tag=f"v{g}", name=f"v_res{g}")
             for g in range(4)]

    # collective bounce buffers (DRAM pool tiles -> Tile tracks deps).
    # Shard = [128 rows, 4096 bf16] per tensor (8 KiB rows), so a
    # gathered chunk reload is ONE per-partition row-gather DMA.
    if SPLIT_CC:
        kt_in = dram_p.tile([128, 4096], BF16, tag="kt_in")
        v_in = dram_p.tile([128, 4096], BF16, tag="v_in")
        if not timing:
            kt_gath = dram_p.tile([1024, 4096], BF16, tag="kt_gath",
                                  addr_space="Shared")
            v_gath = dram_p.tile([1024, 4096], BF16, tag="v_gath",
                                 addr_space="Shared")
        else:
            kt_gath, v_gath = kt_in, v_in
    else:
        kv_in = dram_p.tile([256, 4096], BF16, tag="kv_in")
        if not timing:
            kv_gath = dram_p.tile([2048, 4096], BF16, tag="kv_gath",
                                  addr_space="Shared")
        else:
            kv_gath = kv_in
        kt_in = kv_in[0:128, :]
        v_in = kv_in[128:256, :]
        kt_gath = v_gath = kv_gath

    def load_chunk(dst, src, col):
        nc.gpsimd.indirect_dma_start(
            out=dst[:, :],
            out_offset=None,
            in_=src[:, :],
            in_offset=bass.IndirectOffsetOnAxis(ap=gidx_t[:, col:col + 1],
                                                axis=0),
        )

    # =====================================================================
    # Projections: K^T own -> V own -> fused AllGather -> Q^T + Wo
    # =====================================================================
    with tc.tile_pool(name="pP", bufs=1) as pp, \
         tc.tile_pool(name="wstream", bufs=8) as wsp, \
         tc.tile_pool(name="ps_pmm", bufs=8, space="PSUM") as ps_mm:

        xt_kv = pp.tile([128, 4096], BF16, tag="xt_kv")
        xt_q = pp.tile([128, 4096], BF16, tag="xt_q")
        kt_own = pp.tile([128, 4096], BF16, tag="kt_own")
        v_own = pp.tile([128, 4096], BF16, tag="v_own")

        # first-MM-critical DMAs lead the sync ring: x^T d-slice 0, then
        # the first wk slice arrives inside the K-proj loop below.
        mark("xt_kv")
        xkv3 = xt_kv.rearrange("p (d b) -> p d b", b=512)
        nc.sync.dma_start(xkv3[:, 0, :], x_kvT.ap()[0:128, :])

        # ---- K^T own: kt_own[p, 512m+tok] = relu(K^T)[128m+p, tok]
        mark("kT")
        for half in range(2):
            mms = [ps_mm.tile([128, 512], F32, tag="mm", name=f"mmk{half}_{i}")
                   for i in range(4)]
            for d in range(8):
                wk_d = wsp.tile([128, 512], BF16, tag="w", name=f"wk{half}{d}")
                nc.sync.dma_start(
                    wk_d[:], wk_in.ap()[128 * d:128 * (d + 1),
                                        512 * half:512 * (half + 1)])
                if half == 0 and d < 7:
                    nc.sync.dma_start(xkv3[:, d + 1, :],
                                      x_kvT.ap()[128 * (d + 1):128 * (d + 2), :])
                for mi in range(4):
                    nc.tensor.matmul(
                        mms[mi][:],
                        wk_d[:, 128 * mi:128 * (mi + 1)],
                        xt_kv[:, 512 * d:512 * (d + 1)],
                        start=(d == 0), stop=(d == 7),
                    )
            for mi in range(4):
                m = 4 * half + mi
                nc.scalar.activation(kt_own[:, 512 * m:512 * (m + 1)],
                                     mms[mi][:], Relu)
        nc.scalar.dma_start(kt_in[:, :], kt_own[:, :])

        mark("cc_kt")
        if SPLIT_CC and not timing:
            nc.gpsimd.collective_compute(
                "AllGather",
                mybir.AluOpType.bypass,
                replica_groups=[[0, 1, 2, 3, 4, 5, 6, 7]],
                ins=[kt_in[:, :]],
                outs=[kt_gath[:, :]],
            )

        # ---- V own: v_own[p, 1024t+dv] = relu(V)[128t+p, dv]
        mark("V")
        for h in range(2):
            mms = [ps_mm.tile([128, 512], F32, tag="mm", name=f"mmv{h}_{i}")
                   for i in range(4)]
            for d in range(8):
                wv_d = wsp.tile([128, 512], BF16, tag="w", name=f"wv{h}{d}")
                nc.sync.dma_start(
                    wv_d[:], wv_in.ap()[128 * d:128 * (d + 1),
                                        512 * h:512 * (h + 1)])
                for t in range(4):
                    nc.tensor.matmul(
                        mms[t][:],
                        xt_kv[:, 512 * d + 128 * t:512 * d + 128 * (t + 1)],
                        wv_d[:],
                        start=(d == 0), stop=(d == 7),
                    )
            for t in range(4):
                nc.scalar.activation(
                    v_own[:, 1024 * t + 512 * h:1024 * t + 512 * (h + 1)],
                    mms[t][:], Relu)
        nc.scalar.dma_start(v_in[:, :], v_own[:, :])

        mark("consts")
        nc.sync.dma_start(gidx_t[:], gidx_in.ap())
        nc.sync.dma_start(ident_t[:], ident_in.ap())
        nc.sync.dma_start(mask_t[:], mask_in.ap())

        mark("cc_v")
        if not timing:
            if SPLIT_CC:
                nc.gpsimd.collective_compute(
                    "AllGather",
                    mybir.AluOpType.bypass,
                    replica_groups=[[0, 1, 2, 3, 4, 5, 6, 7]],
                    ins=[v_in[:, :]],
                    outs=[v_gath[:, :]],
                )
            else:
                nc.gpsimd.collective_compute(
                    "AllGather",
                    mybir.AluOpType.bypass,
                    replica_groups=[[0, 1, 2, 3, 4, 5, 6, 7]],
                    ins=[kv_in[:, :]],
                    outs=[kv_gath[:, :]],
                )

        mark("xt_q")
        xq3 = xt_q.rearrange("p (d b) -> p d b", b=512)
        for d in range(8):
            nc.sync.dma_start(xq3[:, d, :],
                              x_qT.ap()[128 * d:128 * (d + 1), :])

        # ---- Q^T own (scaled by 1/sqrt(D))
        mark("qT")
        for half in range(2):
            mms = [ps_mm.tile([128, 512], F32, tag="mm", name=f"mmq{half}_{i}")
                   for i in range(4)]
            for d in range(8):
                wq_d = wsp.tile([128, 512], BF16, tag="w", name=f"wq{half}{d}")
                nc.sync.dma_start(
                    wq_d[:], wq_in.ap()[128 * d:128 * (d + 1),
                                        512 * half:512 * (half + 1)])
                for mi in range(4):
                    nc.tensor.matmul(
                        mms[mi][:],
                        wq_d[:, 128 * mi:128 * (mi + 1)],
                        xt_q[:, 512 * d:512 * (d + 1)],
                        start=(d == 0), stop=(d == 7),
                    )
            for mi in range(4):
                m = 4 * half + mi
                nc.scalar.activation(qt_t[:, 512 * m:512 * (m + 1)],
                                     mms[mi][:], Relu, scale=SCALE)

        mark("woload")
        for d in range(8):
            nc.sync.dma_start(wo_t[:, 1024 * d:1024 * (d + 1)],
                              wo_in.ap()[128 * d:128 * (d + 1), :])

        # gathered chunk loads (gpsimd ring, behind the AG trigger(s)),
        # in consumption order.  Split mode: all kt loads first (they
        # only wait on the early K-AG; a v load would block the queue on
        # the V-AG), then v loads.
        mark("kvload")
        if SPLIT_CC:
            # v loads follow the [0, 2, 3, 1] block processing order
            order = [("kt", 0), ("kt", 1), ("kt", 2), ("kt", 3),
                     ("v", 0), ("v", 2), ("v", 3), ("v", 1)]
        else:
            order = [("kt", 0), ("v", 0), ("kt", 1), ("kt", 2),
                     ("v", 1), ("kt", 3), ("v", 2), ("v", 3)]
        for kind, g in order:
            if kind == "kt":
                load_chunk(kt_res[g], kt_gath, g)
            else:
                load_chunk(v_res[g], v_gath, 4 + g)

    # =====================================================================
    # Attention + output projection, software-pipelined across blocks
    # =====================================================================
    with tc.tile_pool(name="ps_cmm", bufs=2, space="PSUM") as ps_mm, \
         tc.tile_pool(name="ps_ctr", bufs=2, space="PSUM") as ps_tr, \
         tc.tile_pool(name="ps_y", bufs=1, space="PSUM") as ps_y, \
         tc.tile_pool(name="ps_yt", bufs=1, space="PSUM") as ps_yt:

        st = {}

        def emit_e(i):
            mark(f"e{i}")
            if i == 0:
                # the [128,1024] output-staging slot is idle until the first
                # tail and block0's scores die before it -- reuse it so later
                # blocks' copies never wait on block0's transposes.
                e_t = out_p.tile([128, 1024], F32, tag="osb", name="e0")[:, 0:512]
            else:
                # eA holds block2 then block1; eB holds block3 (sized to max)
                e_t = e_p.tile([128, 512 * (i + 1)], F32,
                               tag=("eA" if i in (1, 2) else "eB"),
                               name=f"e{i}")
            st[i] = {"e": e_t}
            negmax = None
            for g in range(i + 1):
                mm = ps_mm.tile([128, 512], F32, tag="mm", name=f"mme{i}{g}")
                ktg = kt_res[g].rearrange("p (d b) -> p d b", b=512)
                for d in range(8):
                    nc.tensor.matmul(
                        mm[:],
                        qt_t[:, 512 * d + 128 * i:512 * d + 128 * (i + 1)],
                        ktg[:, d, :],
                        start=(d == 0), stop=(d == 7),
                    )
                if g == i:
                    nc.vector.tensor_add(e_t[:, 512 * g:512 * (g + 1)],
                                         mm[:], mask_t[:])
                else:
                    nc.vector.tensor_copy(e_t[:, 512 * g:512 * (g + 1)], mm[:])
                # incremental per-chunk negated max, hidden behind the next
                # chunk's score matmuls: negmax = min_g(-max(chunk_g))
                nm_g = st_p.tile([128, 1], F32, tag="nmg", name=f"nm{i}_{g}")
                nc.vector.reduce_max(nm_g[:], e_t[:, 512 * g:512 * (g + 1)],
                                     axis=AX, negate=True)
                if negmax is None:
                    negmax = nm_g
                else:
                    acc = st_p.tile([128, 1], F32, tag="nmacc",
                                    name=f"nma{i}_{g}")
                    nc.vector.tensor_tensor(acc[:], negmax[:], nm_g[:],
                                            op=mybir.AluOpType.min)
                    negmax = acc
            st[i]["negmax"] = negmax

        def emit_softmax(i):
            mark(f"sm{i}")
            e_t = st[i]["e"]
            W = 512 * (i + 1)
            rowsum = st_p.tile([128, 1], F32, tag="rowsum", name=f"rs{i}")
            nc.scalar.activation(e_t[:, 0:W], e_t[:, 0:W], Exp,
                                 bias=st[i]["negmax"][:], scale=1.0,
                                 accum_out=rowsum[:])
            rinv = st_p.tile([128, 1], F32, tag="rinv", name=f"ri{i}")
            nc.vector.reciprocal(rinv[:], rowsum[:])
            st[i]["rinv"] = rinv

        def emit_trav(i):
            mark(f"av{i}")
            e_t = st[i]["e"]
            yps = ps_y.tile([128, 1024], F32, tag="yacc", name=f"y{i}")
            st[i]["yps"] = yps
            for g in range(i + 1):
                trp = ps_tr.tile([128, 512], F32, tag="ctr", name=f"ctr{i}{g}")
                for j in range(4):
                    nc.tensor.transpose(
                        trp[:, 128 * j:128 * (j + 1)],
                        e_t[:, 512 * g + 128 * j:512 * g + 128 * (j + 1)],
                        ident_t[:],
                    )
                pt_t = pt_p.tile([128, 512], BF16, tag="pt", name=f"pt{i}{g}")
                nc.vector.tensor_copy(pt_t[:], trp[:])
                vg = v_res[g].rearrange("p (t b) -> p t b", b=1024)
                for j in range(4):
                    for h in range(2):
                        nc.tensor.matmul(
                            yps[:, 512 * h:512 * (h + 1)],
                            pt_t[:, 128 * j:128 * (j + 1)],
                            vg[:, j, 512 * h:512 * (h + 1)],
                            start=(g == 0 and j == 0),
                            stop=(g == i and j == 3),
                        )

        def emit_tail(i):
            # y stays unnormalized; 1/rowsum is applied as the per-partition
            # scale of the final relu (relu(a*c) = relu(a)*c for c > 0).
            mark(f"tail{i}")
            y_t = y_p.tile([128, 1024], F32, tag="ysb", name=f"ysb{i}")
            nc.vector.tensor_copy(y_t[:, 0:512], st[i]["yps"][:, 0:512])
            nc.vector.tensor_copy(y_t[:, 512:1024], st[i]["yps"][:, 512:1024])
            ytp = ps_yt.tile([128, 1024], F32, tag="ytp", name=f"ytp{i}")
            yt_t = yt_p.tile([128, 1024], BF16, tag="ytsb", name=f"ytsb{i}")
            for d in range(8):
                nc.tensor.transpose(
                    ytp[:, 128 * d:128 * (d + 1)],
                    y_t[:, 128 * d:128 * (d + 1)],
                    ident_t[:],
                )
                if d == 3:
                    # half-granular evacuation: the first out-proj matmuls
                    # can start while the second half still transposes.
                    nc.vector.tensor_copy(yt_t[:, 0:512], ytp[:, 0:512])
            nc.vector.tensor_copy(yt_t[:, 512:1024], ytp[:, 512:1024])
            o_t = out_p.tile([128, 1024], F32, tag="osb", name=f"osb{i}")
            for h in range(2):
                mm = ps_mm.tile([128, 512], F32, tag="mm", name=f"mmo{i}{h}")
                for d in range(8):
                    nc.tensor.matmul(
                        mm[:],
                        yt_t[:, 128 * d:128 * (d + 1)],
                        wo_t[:, 1024 * d + 512 * h:1024 * d + 512 * (h + 1)],
                        start=(d == 0), stop=(d == 7),
                    )
                nc.scalar.activation(o_t[:, 512 * h:512 * (h + 1)], mm[:], Relu,
                                     scale=st[i]["rinv"][:])
                # per-half store: the second half's relu overlaps the first
                # half's writeback, shortening the final-block drain.
                nc.scalar.dma_start(
                    y_out.ap()[128 * i:128 * (i + 1), 512 * h:512 * (h + 1)],
                    o_t[:, 512 * h:512 * (h + 1)])

        # pipelined emission: PE fills softmax bubbles with the next
        # block's score matmuls.  Block order [0, 2, 3, 1]: block 0 first
        # (needs only kt chunk 0 -> earliest start), block 1 last so the
        # exposed serial tail (scores->softmax->AV->out-proj of the final
        # block) is the second-smallest instead of the largest.
        o = [0, 2, 3, 1]
        emit_e(o[0])
        emit_softmax(o[0])
        emit_e(o[1])
        emit_trav(o[0])
        emit_tail(o[0])
        emit_softmax(o[1])
        emit_e(o[2])
        emit_trav(o[1])
        emit_tail(o[1])
        emit_softmax(o[2])
        emit_e(o[3])
        emit_trav(o[2])
        emit_tail(o[2])
        emit_softmax(o[3])
        emit_trav(o[3])
        emit_tail(o[3])

    mark("end")
    for p in reversed(pools):
        p.release()


_PROGRAM_CACHE = {}


def _get_program():
    if "nc" not in _PROGRAM_CACHE:
        _PROGRAM_CACHE["nc"] = _build_program()
    return _PROGRAM_CACHE["nc"]


# ---------------------------------------------------------------------------
# Host-side entry point
# ---------------------------------------------------------------------------


def _bf16(a):
    return np.asarray(np.asarray(a, dtype=np.float32),
                      dtype=ml_dtypes.bfloat16)


def _make_mask(r):
    i = np.arange(128)[:, None]
    j = np.arange(512)[None, :]
    return np.where(j > 128 * r + i, np.float32(-NEG), np.float32(0.0))


def _make_in_maps(x, Wq, Wk, Wv, Wo):
    x = np.asarray(x, dtype=np.float32)
    wq = _bf16(Wq)
    wk = _bf16(Wk)
    wv = _bf16(Wv)
    wo = _bf16(Wo)
    ident = np.eye(128, dtype=np.float32)


    in_maps = []
    for core in range(8):
        b, r = divmod(core, 4)
        xb = x[b]
        x_kvT = np.ascontiguousarray(_bf16(xb[512 * r:512 * (r + 1)]).T)
        chunks = [r, r + 4, r + 8, r + 12]
        x_q = np.concatenate([xb[128 * c:128 * (c + 1)] for c in chunks],
                             axis=0)
        x_qT = np.ascontiguousarray(_bf16(x_q).T)
        # gather rows for the chunk reloads.  Split mode: chunk g of this
        # core's batch sits at rows [128*(4b+g), ...) of each gather
        # buffer.  Fused mode: core shards are [256c, 256c+256) of
        # kv_gath with kt at +0 and v at +128.
        p = np.arange(128, dtype=np.int32)
        gidx = np.empty((128, 8), dtype=np.int32)
        for g in range(4):
            if SPLIT_CC:
                gidx[:, g] = 128 * (4 * b + g) + p
                gidx[:, 4 + g] = 128 * (4 * b + g) + p
            else:
                gidx[:, g] = 256 * (4 * b + g) + p
                gidx[:, 4 + g] = 256 * (4 * b + g) + 128 + p
        in_maps.append({
            "x_kvt": x_kvT, "x_qt": x_qT,
            "wq": wq, "wk": wk, "wv": wv, "wo": wo,
            "mask": _make_mask(r), "ident": ident, "gidx": gidx,
        })
    return in_maps


def kernel(x, Wq, bq, Wk, bk, Wv, bv, Wo, bo, _bench=None):
    nc = _get_program()
    in_maps = _make_in_maps(x, Wq, Wk, Wv, Wo)

    kwargs = dict(_bench or {})
    res = run_bass_kernel_spmd(nc, in_maps, list(range(8)), **kwargs)

    out = np.empty((B, S, D), dtype=np.float32)
    for core in range(8):
        b, r = divmod(core, 4)
        yo = res.results[core]["y_out"]
        for i, c in enumerate([r, r + 4, r + 8, r + 12]):
            out[b, 128 * c:128 * (c + 1), :] = yo[128 * i:128 * (i + 1), :]
    if _bench is not None:
        kernel.last_result = res
    return out


kernel.last_result = None


# ---------------------------------------------------------------------------
# Benchmarking helper (used by test.py only): runs the kernel repeatedly
# through a persistent jitted PJRT executable with device-resident inputs,
# so per-call wall time approximates dispatch-overhead + HW exec time.
# ---------------------------------------------------------------------------


def make_runner(nc, in_maps, chain=1):
    """chain>1 invokes the NEFF that many times inside ONE jitted XLA
    program, feeding iteration i's outputs as iteration i+1's output
    seed buffers (true data dependence -> no CSE, serialized NEFF
    executions).  Per-call dispatch overhead is paid once, so
    (wall(chain=K) - wall(chain=1)) / (K-1) isolates HW exec time."""
    import jax
    from jax.sharding import Mesh, PartitionSpec, NamedSharding
    from jax.experimental.shard_map import shard_map
    from concourse.bass2jax import (
        _bass_exec_p, install_neuronx_cc_hook, partition_id_tensor,
    )

    install_neuronx_cc_hook()
    n_cores = len(in_maps)
    in_names, out_names, out_avals, zero_outs = [], [], [], []
    pname = nc.partition_id_tensor.name if nc.partition_id_tensor else None
    for alloc in nc.m.functions[0].allocations:
        if not isinstance(alloc, mybir.MemoryLocationSet):
            continue
        name = alloc.memorylocations[0].name
        if alloc.kind == "ExternalInput":
            if name != pname:
                in_names.append(name)
        elif alloc.kind == "ExternalOutput":
            shape = tuple(alloc.tensor_shape)
            dtype = mybir.dt.np(alloc.dtype)
            out_names.append(name)
            out_avals.append(jax.core.ShapedArray(shape, dtype))
            zero_outs.append(np.zeros(shape, dtype))
    n_params = len(in_names)
    all_in = list(in_names) + list(out_names)
    if pname:
        all_in.append(pname)

    def _body(*args):
        ins = list(args[:n_params])
        outs = tuple(args[n_params:])
        for _ in range(chain):
            operands = ins + list(outs)
            if pname is not None:
                operands.append(partition_id_tensor())
            outs = tuple(_bass_exec_p.bind(
                *operands, out_avals=tuple(out_avals), in_names=tuple(all_in),
                out_names=tuple(out_names), lowering_input_output_aliases=(),
                sim_require_finite=True, sim_require_nnan=True, nc=nc))
        return outs

    devices = jax.devices()[:n_cores]
    mesh = Mesh(np.asarray(devices), ("core",))
    specs_in = (PartitionSpec("core"),) * (n_params + len(out_names))
    specs_out = (PartitionSpec("core"),) * len(out_names)
    fn = jax.jit(shard_map(_body, mesh=mesh, in_specs=specs_in,
                           out_specs=specs_out, check_rep=False),
                 keep_unused=True)
    sh = NamedSharding(mesh, PartitionSpec("core"))
    concat_in = [np.concatenate([np.asarray(m[n]) for m in in_maps], axis=0)
                 for n in in_names]
    concat_zero = [np.zeros((n_cores * z.shape[0], *z.shape[1:]), z.dtype)
                   for z in zero_outs]
    dev_in = [jax.device_put(a, sh) for a in concat_in]
    dev_zero = [jax.device_put(a, sh) for a in concat_zero]
    return fn, dev_in, dev_zero, out_names


def bench_hw(inputs, iters=60, trials=3):
    """Pipelined-dispatch wall-clock per call for the real 8-core program
    and for a trivial null program; the difference estimates HW exec time."""
    import time
    import jax

    in_maps = _make_in_maps(inputs["x"], inputs["Wq"], inputs["Wk"],
                            inputs["Wv"], inputs["Wo"])

    def null_program():
        nnc = bass.Bass("TRN2", target_bir_lowering=False, debug=False,
                        num_devices=8)
        xi = nnc.dram_tensor("xn", [128, 128], F32, kind="ExternalInput")
        yo = nnc.dram_tensor("yn", [128, 128], F32, kind="ExternalOutput")
        with tile.TileContext(nnc) as tcc:
            with tcc.tile_pool(name="s", bufs=1) as pl:
                t = pl.tile([128, 128], F32, name="t0")
                nnc.sync.dma_start(t[:], xi.ap())
                nnc.sync.dma_start(yo.ap(), t[:])
        _split_waits(nnc)
        return nnc, [{"xn": np.zeros((128, 128), np.float32)}] * 8

    def measure(fn, di, dz):
        out = fn(*di, *dz)
        jax.block_until_ready(out)
        best = float("inf")
        for _ in range(trials):
            t0 = time.perf_counter()
            outs = [fn(*di, *dz) for _ in range(iters)]
            jax.block_until_ready(outs)
            dt = (time.perf_counter() - t0) / iters
            best = min(best, dt)
        return best

    nnc, null_maps = null_program()
    fn0, di0, dz0, _ = make_runner(nnc, null_maps)
    t_null = measure(fn0, di0, dz0)
    fn1, di1, dz1, _ = make_runner(_get_program(), in_maps)
    t_full = measure(fn1, di1, dz1)
    return t_full, t_null
